# revision 15
# baseline (speedup 1.0000x reference)
"""Trainium2 Bass kernel for nn_GAT_skip_forward_15135464751860.

4-layer GATv2 + BN + residual + ELU + linear head on a fixed random graph
(N=20000 nodes, E=480000 edges + self loops), sharded over 8 NeuronCores by
destination node.

Strategy per layer (per core, nodes sharded 2500/core, dst-sorted edges):
  P1  node transform: xl = h @ wl (raw), xr~ = h @ wr + (bl+br) via PE;
      xl shards AllGathered into a full-table in HBM for gathers.
  P2  edge phase per 128-dst block: batched dma_gather of xl[src] and
      xr~[dst] (bf16 rows), z = xl+xr~ on GpSimd, leaky via Prelu on ACT,
      att-dot on DVE (mult + strided tree-reduce), p = exp(e) on ACT,
      wp = [z*p | p], one-hot S_T built by is_equal(dstl, iota), and
      out[i,:] = sum_e S[i,e] wp[e,:] accumulated on the PE into PSUM.
      Softmax denominator rides along as wp column block 256:264.
      out = num/s - xr~ + (bl + conv_bias) (+ residual), since
      sum_e a_e * xl[src_e] = sum_e a_e * z_e - xr~[dst].
  BN  stats via ones-matmul into PSUM, AllReduce, normalize + (ELU).
Head: two small matmuls with a PE transpose in between.
"""
import os, sys

os.environ.setdefault("JAX_PLATFORMS", "")
if "/opt/trn_rl_repo" not in sys.path:
    sys.path.insert(0, "/opt/trn_rl_repo")

import numpy as np
import ml_dtypes
from contextlib import ExitStack

import concourse.bass as bass
import concourse.tile as tile
from concourse import bacc, mybir
from concourse.bass_utils import run_bass_kernel_spmd

BF16 = ml_dtypes.bfloat16

# problem dims
N = 20000
E = 480000
NFEAT = 128
H = 8
C = 32
HC = 256
NCLASS = 10
NCONVS = 3
EPS = 1e-5
SLOPE = 0.2

NCORES = 8
SH = N // NCORES          # 2500 real nodes per core
NT = 20                   # dst blocks / node tiles per core (20*128 = 2560)
SHP = NT * 128            # padded shard rows
NLAYERS = 4

F32 = mybir.dt.float32
BF = mybir.dt.bfloat16
I16 = mybir.dt.int16
AX = mybir.AluOpType
AF = mybir.ActivationFunctionType


# ----------------------------------------------------------------------------
# host-side graph preprocessing
# ----------------------------------------------------------------------------

def _preprocess_edges(edge_index):
    src = np.concatenate([np.asarray(edge_index[0], np.int64), np.arange(N)])
    dst = np.concatenate([np.asarray(edge_index[1], np.int64), np.arange(N)])

    core = dst // SH
    per_core = []
    for c in range(NCORES):
        m = core == c
        s, d = src[m], dst[m]
        o = np.argsort(d, kind="stable")
        s, d = s[o], d[o]
        dloc = d - c * SH
        blk = dloc // 128
        per_core.append((s, dloc, blk))

    # uniform tile count per block across cores (SPMD: one program).
    # need[b] = pad-dst slots that must receive a dummy edge so their softmax
    # denominator stays finite (0 x inf -> NaN otherwise).
    need = np.array([128 - min(128, SH - b * 128) for b in range(NT)], np.int64)
    T = np.zeros(NT, np.int64)
    for c in range(NCORES):
        _, _, blk = per_core[c]
        cnt = np.bincount(blk, minlength=NT)
        T = np.maximum(T, (cnt + need + 127) // 128)
    T = np.maximum(T, 1)
    off = np.zeros(NT + 1, np.int64)
    off[1:] = np.cumsum(T)
    TC = int(off[-1])

    srcw = np.zeros((NCORES, 128, TC * 8), np.int16)
    dstw = np.zeros((NCORES, 128, TC * 8), np.int16)
    dstl = np.full((NCORES, 128, TC), -1.0, np.float32)

    for c in range(NCORES):
        s, dloc, blk = per_core[c]
        for b in range(NT):
            m = blk == b
            sb_, db_ = s[m], dloc[m]
            L = int(T[b]) * 128
            srow = np.zeros(L, np.int64)
            drow = np.zeros(L, np.int64)
            dl = np.full(L, -1.0, np.float32)
            n = len(sb_)
            srow[:n] = (sb_ // SH) * SHP + (sb_ % SH)   # row in AllGathered table
            drow[:n] = db_                              # row in local xr table
            dl[:n] = db_ - b * 128                      # in [0,128)
            nreal = min(128, SH - b * 128)
            for k in range(128 - nreal):                # dummy edges -> pad dsts
                dl[n + k] = nreal + k
            # wrapped int16 idx layout for dma_gather: arr[p, s] = idx[s*16+p],
            # replicated over the 8 Q7 cores (16 partitions * 8)
            sw = srow.reshape(-1, 16).T.astype(np.int16)
            dw = drow.reshape(-1, 16).T.astype(np.int16)
            c0 = int(off[b])
            srcw[c, :, c0 * 8:(c0 + int(T[b])) * 8] = np.tile(sw, (8, 1))
            dstw[c, :, c0 * 8:(c0 + int(T[b])) * 8] = np.tile(dw, (8, 1))
            dstl[c, :, c0:c0 + int(T[b])] = dl.reshape(int(T[b]), 128).T

    return [int(t) for t in T], [int(o) for o in off], TC, srcw, dstw, dstl


# ----------------------------------------------------------------------------
# device program
# ----------------------------------------------------------------------------

def _build_program(T, off, TC):
    nc = bacc.Bacc(None, target_bir_lowering=False, num_devices=NCORES)

    # --- external inputs -----------------------------------------------------
    h0s_d = nc.dram_tensor("h0s", [SHP, NFEAT], BF, kind="ExternalInput")
    srcw_d = nc.dram_tensor("srcw", [128, TC * 8], I16, kind="ExternalInput")
    dstw_d = nc.dram_tensor("dstw", [128, TC * 8], I16, kind="ExternalInput")
    dstl_d = nc.dram_tensor("dstl", [128, TC], BF, kind="ExternalInput")

    wl_d = [nc.dram_tensor(f"wl{l}", [128, 256 if l == 0 else 512], BF, kind="ExternalInput") for l in range(NLAYERS)]
    wr_d = [nc.dram_tensor(f"wr{l}", [128, 256 if l == 0 else 512], BF, kind="ExternalInput") for l in range(NLAYERS)]
    biasr_d = [nc.dram_tensor(f"biasr{l}", [1, 256], BF, kind="ExternalInput") for l in range(NLAYERS)]
    blp_d = [nc.dram_tensor(f"blp{l}", [128, 256], BF, kind="ExternalInput") for l in range(NLAYERS)]
    att_d = [nc.dram_tensor(f"att{l}", [128, 256], BF, kind="ExternalInput") for l in range(NLAYERS)]
    bng_d = [nc.dram_tensor(f"bng{s}", [1, 256], F32, kind="ExternalInput") for s in range(2)]
    bnb_d = [nc.dram_tensor(f"bnb{s}", [1, 256], F32, kind="ExternalInput") for s in range(2)]
    iota_d = nc.dram_tensor("iota", [128, 128], BF, kind="ExternalInput")
    ident_d = nc.dram_tensor("ident", [128, 128], BF, kind="ExternalInput")
    onesc_d = nc.dram_tensor("onesc", [128, 1], BF, kind="ExternalInput")
    onesr_d = nc.dram_tensor("onesr", [1, 128], BF, kind="ExternalInput")
    lin0w_d = nc.dram_tensor("lin0w", [128, 2 * 32], BF, kind="ExternalInput")
    lin0b_d = nc.dram_tensor("lin0b", [1, 32], BF, kind="ExternalInput")
    lin1w_d = nc.dram_tensor("lin1w", [32, 10], BF, kind="ExternalInput")
    lin1b_d = nc.dram_tensor("lin1b", [1, 10], BF, kind="ExternalInput")

    out_d = nc.dram_tensor("out", [SHP, NCLASS], F32, kind="ExternalOutput")

    rg = [list(range(NCORES))]

    with tile.TileContext(nc) as tc, ExitStack() as ctx:
        cst = ctx.enter_context(tc.tile_pool(name="cst", bufs=1))
        sb = ctx.enter_context(tc.tile_pool(name="sb", bufs=2))
        big = ctx.enter_context(tc.tile_pool(name="big", bufs=2))
        res = ctx.enter_context(tc.tile_pool(name="res", bufs=1))
        ps = ctx.enter_context(tc.tile_pool(name="ps", bufs=2, space="PSUM"))
        pstat = ctx.enter_context(tc.tile_pool(name="pstat", bufs=1, space="PSUM"))
        pw = ctx.enter_context(tc.tile_pool(name="pw", bufs=1, space="PSUM"))
        pz = ctx.enter_context(tc.tile_pool(name="pz", bufs=2, space="PSUM"))
        psT = ctx.enter_context(tc.tile_pool(name="psT", bufs=2, space="PSUM"))
        dr = ctx.enter_context(tc.tile_pool(name="dr", bufs=1, space="DRAM"))

        # --- load constants --------------------------------------------------
        def cload(dram, shape, dtype, name):
            t = cst.tile(shape, dtype, name=name)
            nc.sync.dma_start(t[:], dram[:])
            return t

        dstl_s = cload(dstl_d, [128, TC], BF, "dstl_s")
        wl_s = [cload(wl_d[l], [128, 256 if l == 0 else 512], BF, f"wl_s{l}") for l in range(NLAYERS)]
        wr_s = [cload(wr_d[l], [128, 256 if l == 0 else 512], BF, f"wr_s{l}") for l in range(NLAYERS)]
        biasr_s = [cload(biasr_d[l], [1, 256], BF, f"biasr_s{l}") for l in range(NLAYERS)]
        blp_s = [cload(blp_d[l], [128, 256], BF, f"blp_s{l}") for l in range(NLAYERS)]
        att_s = [cload(att_d[l], [128, 256], BF, f"att_s{l}") for l in range(NLAYERS)]
        bng_s = [cload(bng_d[s], [1, 256], F32, f"bng_s{s}") for s in range(2)]
        bnb_s = [cload(bnb_d[s], [1, 256], F32, f"bnb_s{s}") for s in range(2)]
        iota_s = cload(iota_d, [128, 128], BF, "iota_s")
        ident_s = cload(ident_d, [128, 128], BF, "ident_s")
        onesc_s = cload(onesc_d, [128, 1], BF, "onesc_s")
        onesr_s = cload(onesr_d, [1, 128], BF, "onesr_s")
        lin0w_s = cload(lin0w_d, [128, 64], BF, "lin0w_s")
        lin0b_s = cload(lin0b_d, [1, 32], BF, "lin0b_s")
        lin1w_s = cload(lin1w_d, [32, 10], BF, "lin1w_s")
        lin1b_s = cload(lin1b_d, [1, 10], BF, "lin1b_s")
        alpha_s = cst.tile([128, 1], F32, name="alpha_s")
        nc.vector.memset(alpha_s[:], SLOPE)

        h0_dram = h0s_d          # layer-0 node features (bf16, [SHP, 128])
        h_sb = [None] * NT       # SBUF resident h tiles (residual input)

        def node_transform(layer, h_dram, kdim):
            """xl/xr tables for this layer from h_dram [SHP, kdim]."""
            xl_sh = dr.tile([SHP, 256], BF, name=f"xl_sh{layer}")
            xr_tb = dr.tile([SHP, 256], BF, name=f"xr_tb{layer}")
            nhalf = kdim // 128
            for nt in range(NT):
                r0 = nt * 128
                hT = []
                for k in range(nhalf):
                    t = sb.tile([128, 128], BF, name=f"hT{layer}_{nt}_{k}", tag=f"hT{k}")
                    nc.sync.dma_start(out=t[:], in_=h_dram[r0:r0 + 128, k * 128:(k + 1) * 128], transpose=True)
                    hT.append(t)
                xl_ps = pw.tile([128, 256], F32, name=f"xlps{layer}_{nt}", tag="xlps")
                for k in range(nhalf):
                    nc.tensor.matmul(out=xl_ps[:], lhsT=hT[k][:], rhs=wl_s[layer][:, k * 256:(k + 1) * 256],
                                     start=(k == 0), stop=(k == nhalf - 1))
                xl_sb = sb.tile([128, 256], BF, name=f"xlsb{layer}_{nt}", tag="xlsb")
                nc.scalar.copy(xl_sb[:], xl_ps[:])
                nc.sync.dma_start(xl_sh[r0:r0 + 128, :], xl_sb[:])

                xr_ps = pw.tile([128, 256], F32, name=f"xrps{layer}_{nt}", tag="xlps")
                for k in range(nhalf):
                    nc.tensor.matmul(out=xr_ps[:], lhsT=hT[k][:], rhs=wr_s[layer][:, k * 256:(k + 1) * 256],
                                     start=(k == 0), stop=False)
                nc.tensor.matmul(out=xr_ps[:], lhsT=onesr_s[:], rhs=biasr_s[layer][:],
                                 start=False, stop=True)
                xr_sb = sb.tile([128, 256], BF, name=f"xrsb{layer}_{nt}", tag="xrsb")
                nc.scalar.copy(xr_sb[:], xr_ps[:])
                nc.sync.dma_start(xr_tb[r0:r0 + 128, :], xr_sb[:])

            xl_full = dr.tile([NCORES * SHP, 256], BF, name=f"xl_full{layer}", addr_space="Shared")
            nc.gpsimd.collective_compute(
                "AllGather", AX.bypass, replica_groups=rg,
                ins=[xl_sh.opt()], outs=[xl_full.opt()])
            return xl_full, xr_tb

        def edge_phase(layer, xl_full, xr_tb):
            """GATv2 aggregation; returns list of usq tiles ([128,512]: u|u^2)
            and the stats psum tile."""
            st_ps = pstat.tile([1, 512], F32, name=f"stats{layer}", tag="stats")
            usq = [None] * NT
            for b in range(NT):
                Tb = T[b]
                L = Tb * 128
                c0 = off[b]
                siw = sb.tile([128, Tb * 8], I16, name=f"siw{layer}_{b}", tag="siw", bufs=3)
                nc.sync.dma_start(siw[:], srcw_d[:, c0 * 8:(c0 + Tb) * 8])
                xg = big.tile([128, Tb, 256], BF, name=f"xg{layer}_{b}", tag="z", bufs=3)
                nc.gpsimd.dma_gather(
                    out_ap=xg[:], in_ap=xl_full[:], idxs_ap=siw[:],
                    num_idxs=L, num_idxs_reg=L, elem_size=256, single_packet=False)
                xrblk = sb.tile([128, 256], BF, name=f"xrblk{layer}_{b}", tag="xrblk")
                nc.sync.dma_start(xrblk[:], xr_tb[b * 128:(b + 1) * 128, :])
                # one-hot S_T[e, i] = (dstl[e] == i)
                ST = big.tile([128, Tb, 128], BF, name=f"ST{layer}_{b}", tag="ST")
                nc.vector.tensor_tensor(
                    out=ST[:],
                    in0=dstl_s[:, c0:c0 + Tb, None].to_broadcast([128, Tb, 128]),
                    in1=iota_s[:, None, :].to_broadcast([128, Tb, 128]),
                    op=AX.is_equal)
                # per tile: z = S.T @ xr_block + I @ xl_g  on the PE (no xr gather)
                t = big.tile([128, Tb, 256], BF, name=f"t{layer}_{b}", tag="t", bufs=1)
                zs = big.tile([128, Tb, 256], BF, name=f"zs{layer}_{b}", tag="zs")
                for j0 in range(0, Tb, 2):
                    jn = min(2, Tb - j0)
                    Ssb = []
                    for j in range(j0, j0 + jn):
                        sT_ps = psT.tile([128, 128], BF, name=f"sT{layer}_{b}_{j}", tag="sT")
                        nc.tensor.transpose(out=sT_ps[:], in_=ST[:, j, :], identity=ident_s[:])
                        S_sb = sb.tile([128, 128], BF, name=f"Ssb{layer}_{b}_{j}", tag="Ssb")
                        if j % 2 == 0:
                            nc.vector.tensor_copy(S_sb[:], sT_ps[:])
                        else:
                            nc.scalar.copy(S_sb[:], sT_ps[:])
                        Ssb.append(S_sb)
                    z_ps = pz.tile([128, jn, 256], F32, name=f"zps{layer}_{b}_{j0}", tag="zps")
                    for k in range(jn):
                        nc.tensor.matmul(out=z_ps[:, k, :], lhsT=Ssb[k][:], rhs=xrblk[:],
                                         start=True, stop=False, skip_group_check=True)
                        nc.tensor.matmul(out=z_ps[:, k, :], lhsT=ident_s[:], rhs=xg[:, j0 + k, :],
                                         start=False, stop=True, skip_group_check=True)
                    if (j0 // 2) % 2 == 0:
                        nc.vector.tensor_copy(zs[:, j0:j0 + jn, :], z_ps[:])
                    else:
                        nc.scalar.copy(zs[:, j0:j0 + jn, :], z_ps[:])
                nc.scalar.activation(t[:], zs[:], AF.Prelu, bias=0.0, scale=1.0, alpha=alpha_s[:])
                nc.vector.tensor_tensor(
                    out=t[:], in0=t[:],
                    in1=att_s[layer][:, None, :].to_broadcast([128, Tb, 256]), op=AX.mult)
                # e = per-head sum of t  (strided tree reduce)
                t4 = t[:].rearrange("p t (h c) -> p t h c", c=32)
                r16 = big.tile([128, Tb, 8, 16], BF, name=f"r16_{layer}_{b}", tag="r16", bufs=1)
                nc.vector.tensor_tensor(out=r16[:], in0=t4[:, :, :, 0:16], in1=t4[:, :, :, 16:32], op=AX.add)
                r8 = big.tile([128, Tb, 8, 8], BF, name=f"r8_{layer}_{b}", tag="r8", bufs=1)
                nc.vector.tensor_tensor(out=r8[:], in0=r16[:, :, :, 0:8], in1=r16[:, :, :, 8:16], op=AX.add)
                r4 = big.tile([128, Tb, 8, 4], BF, name=f"r4_{layer}_{b}", tag="r4", bufs=1)
                nc.vector.tensor_tensor(out=r4[:], in0=r8[:, :, :, 0:4], in1=r8[:, :, :, 4:8], op=AX.add)
                r2 = big.tile([128, Tb, 8, 2], BF, name=f"r2_{layer}_{b}", tag="r2", bufs=1)
                nc.vector.tensor_tensor(out=r2[:], in0=r4[:, :, :, 0:2], in1=r4[:, :, :, 2:4], op=AX.add)
                e = big.tile([128, Tb, 8], F32, name=f"e{layer}_{b}", tag="e", bufs=1)
                nc.vector.tensor_tensor(out=e[:], in0=r2[:, :, :, 0], in1=r2[:, :, :, 1], op=AX.add)
                # wp = [z * p | p]
                wp = big.tile([128, Tb, 264], BF, name=f"wp{layer}_{b}", tag="wp")
                nc.scalar.activation(wp[:, :, 256:264], e[:], AF.Exp)
                nc.vector.tensor_tensor(
                    out=wp[:, :, 0:256].rearrange("p t (h c) -> p t h c", c=32),
                    in0=zs[:].rearrange("p t (h c) -> p t h c", c=32),
                    in1=wp[:, :, 256:264][:, :, :, None].to_broadcast([128, Tb, 8, 32]),
                    op=AX.mult)
                out_ps = ps.tile([128, 264], F32, name=f"ops{layer}_{b}", tag="out")
                for j in range(Tb):
                    nc.tensor.matmul(out=out_ps[:], lhsT=ST[:, j, :], rhs=wp[:, j, :],
                                     start=(j == 0), stop=(j == Tb - 1))
                # finalize: outn = num/s ; u = outn - xrhat (+ h_res)
                rec = sb.tile([128, 8], F32, name=f"rec{layer}_{b}", tag="rec")
                nc.vector.reciprocal(rec[:], out_ps[:, 256:264])
                us = res.tile([128, 256], BF, name=f"u{layer}_{b}", tag=f"u{b}")
                nc.vector.tensor_tensor(
                    out=us[:].rearrange("p (h c) -> p h c", c=32),
                    in0=out_ps[:, 0:256].rearrange("p (h c) -> p h c", c=32),
                    in1=rec[:, :, None].to_broadcast([128, 8, 32]), op=AX.mult)
                xrh = sb.tile([128, 256], BF, name=f"xrh{layer}_{b}", tag="xrh")
                nc.sync.dma_start(xrh[:], xr_tb[b * 128:(b + 1) * 128, :])
                nc.vector.tensor_sub(us[:], us[:], xrh[:])
                nc.vector.tensor_add(us[:], us[:], blp_s[layer][:])
                if layer > 0:
                    nc.vector.tensor_add(us[:], us[:], h_sb[b][:])
                sq = sb.tile([128, 256], BF, name=f"sq{layer}_{b}", tag="sq")
                nc.scalar.square(sq[:], us[:])
                nreal = 128 if b < NT - 1 else SH - (NT - 1) * 128
                nc.tensor.matmul(out=st_ps[0:1, 0:256], lhsT=onesc_s[0:nreal, :], rhs=us[0:nreal, :],
                                 start=(b == 0), stop=(b == NT - 1), skip_group_check=True)
                nc.tensor.matmul(out=st_ps[0:1, 256:512], lhsT=onesc_s[0:nreal, :], rhs=sq[0:nreal, :],
                                 start=(b == 0), stop=(b == NT - 1), skip_group_check=True)
                usq[b] = us
            return usq, st_ps

        def bn_tail(layer, usq, st_ps, elu):
            """AllReduce stats, normalize (+ELU); returns h dram + fills h_sb."""
            st_sb = sb.tile([1, 512], F32, name=f"stsb{layer}", tag="stsb", bufs=1)
            nc.vector.tensor_copy(st_sb[:], st_ps[:])
            st_in = dr.tile([1, 512], F32, name=f"stin{layer}")
            st_out = dr.tile([1, 512], F32, name=f"stout{layer}", addr_space="Shared")
            nc.gpsimd.dma_start(st_in[:], st_sb[:])
            nc.gpsimd.collective_compute(
                "AllReduce", AX.add, replica_groups=rg,
                ins=[st_in.opt()], outs=[st_out.opt()])
            st2 = sb.tile([1, 512], F32, name=f"st2{layer}", tag="stsb", bufs=1)
            nc.gpsimd.dma_start(st2[:], st_out[:])

            gi = 0 if layer == 0 else 1
            ab = sb.tile([1, 512], F32, name=f"ab{layer}", tag="ab", bufs=1)   # A | B
            mu = sb.tile([1, 256], F32, name=f"mu{layer}", tag="mu", bufs=1)
            nc.vector.tensor_scalar_mul(mu[:], st2[:, 0:256], 1.0 / N)
            var = sb.tile([1, 256], F32, name=f"var{layer}", tag="var", bufs=1)
            nc.vector.tensor_scalar_mul(var[:], st2[:, 256:512], 1.0 / N)
            mu2 = sb.tile([1, 256], F32, name=f"mu2{layer}", tag="mu2", bufs=1)
            nc.vector.tensor_tensor(out=mu2[:], in0=mu[:], in1=mu[:], op=AX.mult)
            nc.vector.tensor_sub(var[:], var[:], mu2[:])
            nc.vector.tensor_scalar_add(var[:], var[:], EPS)
            # rsqrt = exp(-0.5 * ln(var))  (stays in the ln/exp ACT table set)
            lnv = sb.tile([1, 256], F32, name=f"lnv{layer}", tag="lnv", bufs=1)
            nc.scalar.activation(lnv[:], var[:], AF.Ln)
            rs = sb.tile([1, 256], F32, name=f"rs{layer}", tag="rs", bufs=1)
            nc.scalar.activation(rs[:], lnv[:], AF.Exp, bias=0.0, scale=-0.5)
            nc.vector.tensor_tensor(out=ab[:, 0:256], in0=rs[:], in1=bng_s[gi][:], op=AX.mult)
            nc.vector.tensor_tensor(out=mu2[:], in0=mu[:], in1=ab[:, 0:256], op=AX.mult)
            nc.vector.tensor_tensor(out=ab[:, 256:512], in0=bnb_s[gi][:], in1=mu2[:], op=AX.subtract)
            ab_bc = sb.tile([128, 512], F32, name=f"abbc{layer}", tag="abbc", bufs=1)
            nc.gpsimd.partition_broadcast(ab_bc[:], ab[:])

            h_dram = dr.tile([SHP, 256], BF, name=f"h{layer}")
            for b in range(NT):
                y = res.tile([128, 256], BF, name=f"h{layer}_{b}", tag=f"h{layer % 2}_{b}")
                nc.vector.tensor_tensor(out=y[:], in0=usq[b][:], in1=ab_bc[:, 0:256], op=AX.mult)
                nc.vector.tensor_add(y[:], y[:], ab_bc[:, 256:512])
                if elu:
                    ymin = sb.tile([128, 256], BF, name=f"ymin{layer}_{b}", tag="ymin")
                    nc.vector.tensor_scalar_min(ymin[:], y[:], 0.0)
                    expn = sb.tile([128, 256], BF, name=f"expn{layer}_{b}", tag="expn")
                    nc.scalar.activation(expn[:], ymin[:], AF.Exp)
                    nc.scalar.activation(y[:], y[:], AF.Relu)
                    nc.vector.tensor_add(y[:], y[:], expn[:])
                    nc.vector.tensor_scalar_add(y[:], y[:], -1.0)
                h_sb[b] = y
                nc.sync.dma_start(h_dram[b * 128:(b + 1) * 128, :], y[:])
            return h_dram

        # ---------------- main network ----------------
        h_dram = h0_dram
        kdim = NFEAT
        for layer in range(NLAYERS):
            xl_full, xr_tb = node_transform(layer, h_dram, kdim)
            usq, st_ps = edge_phase(layer, xl_full, xr_tb)
            h_dram = bn_tail(layer, usq, st_ps, elu=(layer > 0))
            kdim = 256

        # ---------------- head ----------------
        for nt in range(NT):
            r0 = nt * 128
            hT = []
            for k in range(2):
                t = sb.tile([128, 128], BF, name=f"hTh_{nt}_{k}", tag=f"hT{k}")
                nc.sync.dma_start(out=t[:], in_=h_dram[r0:r0 + 128, k * 128:(k + 1) * 128], transpose=True)
                hT.append(t)
            y1ps = ps.tile([128, 32], F32, name=f"y1ps{nt}", tag="out")
            for k in range(2):
                nc.tensor.matmul(out=y1ps[:], lhsT=hT[k][:], rhs=lin0w_s[:, k * 32:(k + 1) * 32],
                                 start=(k == 0), stop=False)
            nc.tensor.matmul(out=y1ps[:], lhsT=onesr_s[:], rhs=lin0b_s[:], start=False, stop=True)
            y1 = sb.tile([128, 32], BF, name=f"y1_{nt}", tag="y1")
            nc.scalar.copy(y1[:], y1ps[:])
            ymin = sb.tile([128, 32], BF, name=f"hymin{nt}", tag="hymin")
            nc.vector.tensor_scalar_min(ymin[:], y1[:], 0.0)
            expn = sb.tile([128, 32], BF, name=f"hexpn{nt}", tag="hexpn")
            nc.scalar.activation(expn[:], ymin[:], AF.Exp)
            y1e = sb.tile([128, 32], BF, name=f"y1e_{nt}", tag="y1e")
            nc.scalar.activation(y1e[:], y1[:], AF.Relu)
            nc.vector.tensor_add(y1e[:], y1e[:], expn[:])
            nc.vector.tensor_scalar_add(y1e[:], y1e[:], -1.0)
            y1T_ps = ps.tile([32, 128], BF, name=f"y1Tps{nt}", tag="out")
            nc.tensor.transpose(out=y1T_ps[:], in_=y1e[:], identity=ident_s[:])
            y1T = sb.tile([32, 128], BF, name=f"y1T_{nt}", tag="y1T")
            nc.vector.tensor_copy(y1T[:], y1T_ps[:])
            y2ps = ps.tile([128, 10], F32, name=f"y2ps{nt}", tag="out")
            nc.tensor.matmul(out=y2ps[:], lhsT=y1T[:], rhs=lin1w_s[:], start=True, stop=False)
            nc.tensor.matmul(out=y2ps[:], lhsT=onesr_s[:], rhs=lin1b_s[:], start=False, stop=True)
            outf = sb.tile([128, 10], F32, name=f"outf{nt}", tag="outf")
            nc.scalar.copy(outf[:], y2ps[:])
            nc.sync.dma_start(out_d[r0:r0 + 128, :], outf[:])

    nc.finalize()
    return nc


# ----------------------------------------------------------------------------
# host wrapper
# ----------------------------------------------------------------------------

_CACHE = {}


def _prep(inputs):
    x = np.asarray(inputs["x"], np.float32)
    ei = np.asarray(inputs["edge_index"])
    T, off, TC, srcw, dstw, dstl = _preprocess_edges(ei)

    f = lambda k: np.asarray(inputs[k], np.float32)

    # BN0 on the host (depends only on inputs)
    mu = x.mean(0, dtype=np.float64)
    var = ((x.astype(np.float64) - mu) ** 2).mean(0)
    h0 = ((x - mu.astype(np.float32)) / np.sqrt(var + EPS).astype(np.float32)
          * f("norm0_g") + f("norm0_b")).astype(np.float32)

    def pack_w(w):  # [256, 256] -> [128, 512]
        return np.concatenate([w[:128], w[128:]], axis=1)

    wl = [f("conv0_wl")] + [pack_w(f("convs_wl")[i]) for i in range(NCONVS)]
    wr = [f("conv0_wr")] + [pack_w(f("convs_wr")[i]) for i in range(NCONVS)]
    biasr = [(f("conv0_bl") + f("conv0_br"))[None]] + \
            [(f("convs_bl")[i] + f("convs_br")[i])[None] for i in range(NCONVS)]
    blp = [(f("conv0_bl") + f("conv0_bias"))[None]] + \
          [(f("convs_bl")[i] + f("convs_bias")[i])[None] for i in range(NCONVS)]
    att = [f("conv0_att").reshape(1, 256)] + \
          [f("convs_att")[i].reshape(1, 256) for i in range(NCONVS)]

    bf = lambda a: np.ascontiguousarray(a, np.float32).astype(BF16)
    com = {}
    for l in range(NLAYERS):
        com[f"wl{l}"] = bf(wl[l])
        com[f"wr{l}"] = bf(wr[l])
        com[f"biasr{l}"] = bf(biasr[l])
        com[f"blp{l}"] = bf(np.broadcast_to(blp[l], (128, 256)))
        com[f"att{l}"] = bf(np.broadcast_to(att[l], (128, 256)))
    com["bng0"], com["bnb0"] = f("norm1_g")[None].copy(), f("norm1_b")[None].copy()
    com["bng1"], com["bnb1"] = f("norm2_g")[None].copy(), f("norm2_b")[None].copy()
    com["iota"] = bf(np.broadcast_to(np.arange(128, dtype=np.float32)[None], (128, 128)))
    com["ident"] = bf(np.eye(128, dtype=np.float32))
    com["onesc"] = bf(np.ones((128, 1), np.float32))
    com["onesr"] = bf(np.ones((1, 128), np.float32))
    com["lin0w"] = bf(np.concatenate([f("lin0_w")[:128], f("lin0_w")[128:]], axis=1))
    com["lin0b"] = bf(f("lin0_b")[None])
    com["lin1w"] = bf(f("lin1_w"))
    com["lin1b"] = bf(f("lin1_b")[None])

    in_maps = []
    for c in range(NCORES):
        h0c = np.zeros((SHP, NFEAT), np.float32)
        h0c[:SH] = h0[c * SH:(c + 1) * SH]
        m = dict(com)
        m["h0s"] = bf(h0c)
        m["srcw"] = srcw[c]
        m["dstw"] = dstw[c]
        m["dstl"] = dstl[c].astype(BF16)
        in_maps.append(m)
    return (tuple(T), tuple(off), TC), in_maps


def run(inputs, trace=False, trace_kwargs=None, **extra):
    key, in_maps = _prep(inputs)
    if key not in _CACHE:
        _CACHE[key] = _build_program(list(key[0]), list(key[1]), key[2])
    nc = _CACHE[key]
    kw = dict(extra)
    if trace:
        kw.update(trace=True, trace_kwargs=trace_kwargs or {})
    r = run_bass_kernel_spmd(nc, in_maps, list(range(NCORES)), **kw)
    out = np.empty((N, NCLASS), np.float32)
    for c in range(NCORES):
        out[c * SH:(c + 1) * SH] = r.results[c]["out"][:SH]
    return out, r


def kernel(**inputs) -> np.ndarray:
    out, _ = run(inputs, trace=False)
    return out


# revision 16
# speedup vs baseline: 1.3695x; 1.3695x over previous
"""Trainium2 Bass kernel for nn_GAT_skip_forward_15135464751860.

4-layer GATv2 + BN + residual + ELU + linear head on a fixed random graph
(N=20000 nodes, E=480000 edges + self loops), sharded over 8 NeuronCores by
destination node.

Strategy per layer (per core, nodes sharded 2500/core, dst-sorted edges):
  P1  node transform: xl = h @ wl (raw), xr~ = h @ wr + (bl+br) via PE;
      xl shards AllGathered into a full-table in HBM for gathers.
  P2  edge phase per 128-dst block: batched dma_gather of xl[src] and
      xr~[dst] (bf16 rows), z = xl+xr~ on GpSimd, leaky via Prelu on ACT,
      att-dot on DVE (mult + strided tree-reduce), p = exp(e) on ACT,
      wp = [z*p | p], one-hot S_T built by is_equal(dstl, iota), and
      out[i,:] = sum_e S[i,e] wp[e,:] accumulated on the PE into PSUM.
      Softmax denominator rides along as wp column block 256:264.
      out = num/s - xr~ + (bl + conv_bias) (+ residual), since
      sum_e a_e * xl[src_e] = sum_e a_e * z_e - xr~[dst].
  BN  stats via ones-matmul into PSUM, AllReduce, normalize + (ELU).
Head: two small matmuls with a PE transpose in between.
"""
import os, sys

os.environ.setdefault("JAX_PLATFORMS", "")
if "/opt/trn_rl_repo" not in sys.path:
    sys.path.insert(0, "/opt/trn_rl_repo")

import numpy as np
import ml_dtypes
from contextlib import ExitStack

import concourse.bass as bass
import concourse.tile as tile
from concourse import bacc, mybir
from concourse.bass_utils import run_bass_kernel_spmd

BF16 = ml_dtypes.bfloat16

# problem dims
N = 20000
E = 480000
NFEAT = 128
H = 8
C = 32
HC = 256
NCLASS = 10
NCONVS = 3
EPS = 1e-5
SLOPE = 0.2

NCORES = 8
SH = N // NCORES          # 2500 real nodes per core
NT = 20                   # dst blocks / node tiles per core (20*128 = 2560)
SHP = NT * 128            # padded shard rows
NLAYERS = 4

F32 = mybir.dt.float32
BF = mybir.dt.bfloat16
I16 = mybir.dt.int16
AX = mybir.AluOpType
AF = mybir.ActivationFunctionType


# ----------------------------------------------------------------------------
# host-side graph preprocessing
# ----------------------------------------------------------------------------

def _preprocess_edges(edge_index):
    src = np.concatenate([np.asarray(edge_index[0], np.int64), np.arange(N)])
    dst = np.concatenate([np.asarray(edge_index[1], np.int64), np.arange(N)])

    core = dst // SH
    per_core = []
    for c in range(NCORES):
        m = core == c
        s, d = src[m], dst[m]
        o = np.argsort(d, kind="stable")
        s, d = s[o], d[o]
        dloc = d - c * SH
        blk = dloc // 128
        per_core.append((s, dloc, blk))

    # uniform tile count per block across cores (SPMD: one program).
    # need[b] = pad-dst slots that must receive a dummy edge so their softmax
    # denominator stays finite (0 x inf -> NaN otherwise).
    need = np.array([128 - min(128, SH - b * 128) for b in range(NT)], np.int64)
    T = np.zeros(NT, np.int64)
    for c in range(NCORES):
        _, _, blk = per_core[c]
        cnt = np.bincount(blk, minlength=NT)
        T = np.maximum(T, (cnt + need + 127) // 128)
    T = np.maximum(T, 1)
    off = np.zeros(NT + 1, np.int64)
    off[1:] = np.cumsum(T)
    TC = int(off[-1])

    srcw = np.zeros((NCORES, 128, TC * 8), np.int16)
    dstw = np.zeros((NCORES, 128, TC * 8), np.int16)
    dstl = np.full((NCORES, 128, TC), -1.0, np.float32)

    for c in range(NCORES):
        s, dloc, blk = per_core[c]
        for b in range(NT):
            m = blk == b
            sb_, db_ = s[m], dloc[m]
            L = int(T[b]) * 128
            srow = np.zeros(L, np.int64)
            drow = np.zeros(L, np.int64)
            dl = np.full(L, -1.0, np.float32)
            n = len(sb_)
            srow[:n] = (sb_ // SH) * SHP + (sb_ % SH)   # row in AllGathered table
            drow[:n] = db_                              # row in local xr table
            dl[:n] = db_ - b * 128                      # in [0,128)
            nreal = min(128, SH - b * 128)
            for k in range(128 - nreal):                # dummy edges -> pad dsts
                dl[n + k] = nreal + k
            # wrapped int16 idx layout for dma_gather: arr[p, s] = idx[s*16+p],
            # replicated over the 8 Q7 cores (16 partitions * 8)
            sw = srow.reshape(-1, 16).T.astype(np.int16)
            dw = drow.reshape(-1, 16).T.astype(np.int16)
            c0 = int(off[b])
            srcw[c, :, c0 * 8:(c0 + int(T[b])) * 8] = np.tile(sw, (8, 1))
            dstw[c, :, c0 * 8:(c0 + int(T[b])) * 8] = np.tile(dw, (8, 1))
            dstl[c, :, c0:c0 + int(T[b])] = dl.reshape(int(T[b]), 128).T

    return [int(t) for t in T], [int(o) for o in off], TC, srcw, dstw, dstl


# ----------------------------------------------------------------------------
# device program
# ----------------------------------------------------------------------------

def _build_program(T, off, TC):
    nc = bacc.Bacc(None, target_bir_lowering=False, num_devices=NCORES)

    # --- external inputs -----------------------------------------------------
    h0s_d = nc.dram_tensor("h0s", [SHP, NFEAT], BF, kind="ExternalInput")
    srcw_d = nc.dram_tensor("srcw", [128, TC * 8], I16, kind="ExternalInput")
    dstw_d = nc.dram_tensor("dstw", [128, TC * 8], I16, kind="ExternalInput")
    dstl_d = nc.dram_tensor("dstl", [128, TC], BF, kind="ExternalInput")

    wl_d = [nc.dram_tensor(f"wl{l}", [128, 256 if l == 0 else 512], BF, kind="ExternalInput") for l in range(NLAYERS)]
    wr_d = [nc.dram_tensor(f"wr{l}", [128, 256 if l == 0 else 512], BF, kind="ExternalInput") for l in range(NLAYERS)]
    biasr_d = [nc.dram_tensor(f"biasr{l}", [1, 256], BF, kind="ExternalInput") for l in range(NLAYERS)]
    blp_d = [nc.dram_tensor(f"blp{l}", [128, 256], BF, kind="ExternalInput") for l in range(NLAYERS)]
    att_d = [nc.dram_tensor(f"att{l}", [128, 256], BF, kind="ExternalInput") for l in range(NLAYERS)]
    bng_d = [nc.dram_tensor(f"bng{s}", [1, 256], F32, kind="ExternalInput") for s in range(2)]
    bnb_d = [nc.dram_tensor(f"bnb{s}", [1, 256], F32, kind="ExternalInput") for s in range(2)]
    iota_d = nc.dram_tensor("iota", [128, 128], BF, kind="ExternalInput")
    ident_d = nc.dram_tensor("ident", [128, 128], BF, kind="ExternalInput")
    onesc_d = nc.dram_tensor("onesc", [128, 1], BF, kind="ExternalInput")
    onesr_d = nc.dram_tensor("onesr", [1, 128], BF, kind="ExternalInput")
    lin0w_d = nc.dram_tensor("lin0w", [128, 2 * 32], BF, kind="ExternalInput")
    lin0b_d = nc.dram_tensor("lin0b", [1, 32], BF, kind="ExternalInput")
    lin1w_d = nc.dram_tensor("lin1w", [32, 10], BF, kind="ExternalInput")
    lin1b_d = nc.dram_tensor("lin1b", [1, 10], BF, kind="ExternalInput")

    out_d = nc.dram_tensor("out", [SHP, NCLASS], F32, kind="ExternalOutput")

    rg = [list(range(NCORES))]

    with tile.TileContext(nc) as tc, ExitStack() as ctx:
        cst = ctx.enter_context(tc.tile_pool(name="cst", bufs=1))
        sb = ctx.enter_context(tc.tile_pool(name="sb", bufs=2))
        big = ctx.enter_context(tc.tile_pool(name="big", bufs=2))
        res = ctx.enter_context(tc.tile_pool(name="res", bufs=1))
        ps = ctx.enter_context(tc.tile_pool(name="ps", bufs=2, space="PSUM"))
        pstat = ctx.enter_context(tc.tile_pool(name="pstat", bufs=1, space="PSUM"))
        pw = ctx.enter_context(tc.tile_pool(name="pw", bufs=1, space="PSUM"))
        pz = ctx.enter_context(tc.tile_pool(name="pz", bufs=2, space="PSUM"))
        psT = ctx.enter_context(tc.tile_pool(name="psT", bufs=2, space="PSUM"))
        # 2+1+1+2+2 = 8 PSUM banks
        dr = ctx.enter_context(tc.tile_pool(name="dr", bufs=1, space="DRAM"))

        # --- load constants --------------------------------------------------
        def cload(dram, shape, dtype, name):
            t = cst.tile(shape, dtype, name=name)
            nc.sync.dma_start(t[:], dram[:])
            return t

        dstl_s = cload(dstl_d, [128, TC], BF, "dstl_s")
        wl_s = [cload(wl_d[l], [128, 256 if l == 0 else 512], BF, f"wl_s{l}") for l in range(NLAYERS)]
        wr_s = [cload(wr_d[l], [128, 256 if l == 0 else 512], BF, f"wr_s{l}") for l in range(NLAYERS)]
        biasr_s = [cload(biasr_d[l], [1, 256], BF, f"biasr_s{l}") for l in range(NLAYERS)]
        blp_s = [cload(blp_d[l], [128, 256], BF, f"blp_s{l}") for l in range(NLAYERS)]
        att_s = [cload(att_d[l], [128, 256], BF, f"att_s{l}") for l in range(NLAYERS)]
        bng_s = [cload(bng_d[s], [1, 256], F32, f"bng_s{s}") for s in range(2)]
        bnb_s = [cload(bnb_d[s], [1, 256], F32, f"bnb_s{s}") for s in range(2)]
        iota_s = cload(iota_d, [128, 128], BF, "iota_s")
        ident_s = cload(ident_d, [128, 128], BF, "ident_s")
        onesc_s = cload(onesc_d, [128, 1], BF, "onesc_s")
        onesr_s = cload(onesr_d, [1, 128], BF, "onesr_s")
        lin0w_s = cload(lin0w_d, [128, 64], BF, "lin0w_s")
        lin0b_s = cload(lin0b_d, [1, 32], BF, "lin0b_s")
        lin1w_s = cload(lin1w_d, [32, 10], BF, "lin1w_s")
        lin1b_s = cload(lin1b_d, [1, 10], BF, "lin1b_s")
        alpha_s = cst.tile([128, 1], F32, name="alpha_s")
        nc.vector.memset(alpha_s[:], SLOPE)

        h0_dram = h0s_d          # layer-0 node features (bf16, [SHP, 128])
        h_sb = [None] * NT       # SBUF resident h tiles (residual input)

        def node_transform(layer, h_dram, kdim):
            """xl/xr tables for this layer from h_dram [SHP, kdim]."""
            xl_sh = dr.tile([SHP, 256], BF, name=f"xl_sh{layer}")
            xr_tb = dr.tile([SHP, 256], BF, name=f"xr_tb{layer}")
            nhalf = kdim // 128
            for nt in range(NT):
                r0 = nt * 128
                hT = []
                for k in range(nhalf):
                    t = sb.tile([128, 128], BF, name=f"hT{layer}_{nt}_{k}", tag=f"hT{k}")
                    nc.sync.dma_start(out=t[:], in_=h_dram[r0:r0 + 128, k * 128:(k + 1) * 128], transpose=True)
                    hT.append(t)
                xl_ps = pw.tile([128, 256], F32, name=f"xlps{layer}_{nt}", tag="xlps")
                for k in range(nhalf):
                    nc.tensor.matmul(out=xl_ps[:], lhsT=hT[k][:], rhs=wl_s[layer][:, k * 256:(k + 1) * 256],
                                     start=(k == 0), stop=(k == nhalf - 1))
                xl_sb = sb.tile([128, 256], BF, name=f"xlsb{layer}_{nt}", tag="xlsb")
                nc.scalar.copy(xl_sb[:], xl_ps[:])
                nc.sync.dma_start(xl_sh[r0:r0 + 128, :], xl_sb[:])

                xr_ps = pw.tile([128, 256], F32, name=f"xrps{layer}_{nt}", tag="xlps")
                for k in range(nhalf):
                    nc.tensor.matmul(out=xr_ps[:], lhsT=hT[k][:], rhs=wr_s[layer][:, k * 256:(k + 1) * 256],
                                     start=(k == 0), stop=False)
                nc.tensor.matmul(out=xr_ps[:], lhsT=onesr_s[:], rhs=biasr_s[layer][:],
                                 start=False, stop=True)
                xr_sb = sb.tile([128, 256], BF, name=f"xrsb{layer}_{nt}", tag="xrsb")
                nc.scalar.copy(xr_sb[:], xr_ps[:])
                nc.sync.dma_start(xr_tb[r0:r0 + 128, :], xr_sb[:])

            xl_full = dr.tile([NCORES * SHP, 256], BF, name=f"xl_full{layer}", addr_space="Shared")
            nc.gpsimd.collective_compute(
                "AllGather", AX.bypass, replica_groups=rg,
                ins=[xl_sh.opt()], outs=[xl_full.opt()])
            return xl_full, xr_tb

        def edge_phase(layer, xl_full, xr_tb):
            """GATv2 aggregation; returns list of usq tiles ([128,512]: u|u^2)
            and the stats psum tile."""
            st_ps = pstat.tile([1, 512], F32, name=f"stats{layer}", tag="stats")
            usq = [None] * NT
            for b in range(NT):
                Tb = T[b]
                L = Tb * 128
                c0 = off[b]
                siw = sb.tile([128, Tb * 8], I16, name=f"siw{layer}_{b}", tag="siw", bufs=3)
                nc.sync.dma_start(siw[:], srcw_d[:, c0 * 8:(c0 + Tb) * 8])
                xg = big.tile([128, Tb, 256], BF, name=f"xg{layer}_{b}", tag="z", bufs=3)
                nc.gpsimd.dma_gather(
                    out_ap=xg[:], in_ap=xl_full[:], idxs_ap=siw[:],
                    num_idxs=L, num_idxs_reg=L, elem_size=256, single_packet=False)
                xrblk = sb.tile([128, 256], BF, name=f"xrblk{layer}_{b}", tag="xrblk")
                nc.sync.dma_start(xrblk[:], xr_tb[b * 128:(b + 1) * 128, :])
                # one-hot S_T[e, i] = (dstl[e] == i)
                ST = big.tile([128, Tb, 128], BF, name=f"ST{layer}_{b}", tag="ST")
                nc.vector.tensor_tensor(
                    out=ST[:],
                    in0=dstl_s[:, c0:c0 + Tb, None].to_broadcast([128, Tb, 128]),
                    in1=iota_s[:, None, :].to_broadcast([128, Tb, 128]),
                    op=AX.is_equal)
                # per tile: z = S.T @ xr_block + I @ xl_g  on the PE (no xr gather)
                t = big.tile([128, Tb, 256], BF, name=f"t{layer}_{b}", tag="t")
                for j in range(Tb):
                    sT_ps = psT.tile([128, 128], BF, name=f"sT{layer}_{b}_{j}", tag="sT")
                    nc.tensor.transpose(out=sT_ps[:], in_=ST[:, j, :], identity=ident_s[:])
                    S_sb = sb.tile([128, 128], BF, name=f"Ssb{layer}_{b}_{j}", tag="Ssb")
                    if j % 2 == 0:
                        nc.vector.tensor_copy(S_sb[:], sT_ps[:])
                    else:
                        nc.scalar.copy(S_sb[:], sT_ps[:])
                    z_ps = pz.tile([128, 256], F32, name=f"zps{layer}_{b}_{j}", tag="zps")
                    nc.tensor.matmul(out=z_ps[:], lhsT=S_sb[:], rhs=xrblk[:], start=True, stop=False)
                    nc.tensor.matmul(out=z_ps[:], lhsT=ident_s[:], rhs=xg[:, j, :], start=False, stop=True)
                    # leaky(z) straight out of PSUM; z itself is never needed again:
                    # sum_e a_e xl[src_e] = num/s because the xr part cancels (sum a = 1)
                    nc.scalar.activation(t[:, j, :], z_ps[:], AF.Prelu, bias=0.0, scale=1.0, alpha=alpha_s[:])
                nc.vector.tensor_tensor(
                    out=t[:], in0=t[:],
                    in1=att_s[layer][:, None, :].to_broadcast([128, Tb, 256]), op=AX.mult)
                # e = per-head sum of t  (strided tree reduce)
                t4 = t[:].rearrange("p t (h c) -> p t h c", c=32)
                r16 = big.tile([128, Tb, 8, 16], BF, name=f"r16_{layer}_{b}", tag="r16", bufs=1)
                nc.vector.tensor_tensor(out=r16[:], in0=t4[:, :, :, 0:16], in1=t4[:, :, :, 16:32], op=AX.add)
                r8 = big.tile([128, Tb, 8, 8], BF, name=f"r8_{layer}_{b}", tag="r8", bufs=1)
                nc.vector.tensor_tensor(out=r8[:], in0=r16[:, :, :, 0:8], in1=r16[:, :, :, 8:16], op=AX.add)
                r4 = big.tile([128, Tb, 8, 4], BF, name=f"r4_{layer}_{b}", tag="r4", bufs=1)
                nc.vector.tensor_tensor(out=r4[:], in0=r8[:, :, :, 0:4], in1=r8[:, :, :, 4:8], op=AX.add)
                r2 = big.tile([128, Tb, 8, 2], BF, name=f"r2_{layer}_{b}", tag="r2", bufs=1)
                nc.vector.tensor_tensor(out=r2[:], in0=r4[:, :, :, 0:2], in1=r4[:, :, :, 2:4], op=AX.add)
                e = big.tile([128, Tb, 8], F32, name=f"e{layer}_{b}", tag="e", bufs=1)
                nc.vector.tensor_tensor(out=e[:], in0=r2[:, :, :, 0], in1=r2[:, :, :, 1], op=AX.add)
                # wp = [xl_g * p | p]   (xr part of the numerator cancels with -xr~)
                wp = big.tile([128, Tb, 264], BF, name=f"wp{layer}_{b}", tag="wp")
                p_sb = big.tile([128, Tb, 8], BF, name=f"p{layer}_{b}", tag="p", bufs=2)
                nc.scalar.activation(p_sb[:], e[:], AF.Exp)
                nc.vector.tensor_copy(wp[:, :, 256:264], p_sb[:])
                nc.vector.tensor_tensor(
                    out=wp[:, :, 0:256].rearrange("p t (h c) -> p t h c", c=32),
                    in0=xg[:].rearrange("p t (h c) -> p t h c", c=32),
                    in1=p_sb[:, :, :, None].to_broadcast([128, Tb, 8, 32]),
                    op=AX.mult)
                out_ps = ps.tile([128, 264], F32, name=f"ops{layer}_{b}", tag="out")
                for j in range(Tb):
                    nc.tensor.matmul(out=out_ps[:], lhsT=ST[:, j, :], rhs=wp[:, j, :],
                                     start=(j == 0), stop=(j == Tb - 1))
                # finalize: outn = num/s ; u = outn - xrhat (+ h_res)
                rec = sb.tile([128, 8], F32, name=f"rec{layer}_{b}", tag="rec")
                nc.vector.reciprocal(rec[:], out_ps[:, 256:264])
                us = res.tile([128, 256], BF, name=f"u{layer}_{b}", tag=f"u{b}")
                nc.vector.tensor_tensor(
                    out=us[:].rearrange("p (h c) -> p h c", c=32),
                    in0=out_ps[:, 0:256].rearrange("p (h c) -> p h c", c=32),
                    in1=rec[:, :, None].to_broadcast([128, 8, 32]), op=AX.mult)
                nc.vector.tensor_add(us[:], us[:], blp_s[layer][:])
                if layer > 0:
                    nc.vector.tensor_add(us[:], us[:], h_sb[b][:])
                sq = sb.tile([128, 256], BF, name=f"sq{layer}_{b}", tag="sq")
                nc.scalar.square(sq[:], us[:])
                nreal = 128 if b < NT - 1 else SH - (NT - 1) * 128
                nc.tensor.matmul(out=st_ps[0:1, 0:256], lhsT=onesc_s[0:nreal, :], rhs=us[0:nreal, :],
                                 start=(b == 0), stop=(b == NT - 1), skip_group_check=True)
                nc.tensor.matmul(out=st_ps[0:1, 256:512], lhsT=onesc_s[0:nreal, :], rhs=sq[0:nreal, :],
                                 start=(b == 0), stop=(b == NT - 1), skip_group_check=True)
                usq[b] = us
            return usq, st_ps

        def bn_tail(layer, usq, st_ps, elu):
            """AllReduce stats, normalize (+ELU); returns h dram + fills h_sb."""
            st_sb = sb.tile([1, 512], F32, name=f"stsb{layer}", tag="stsb", bufs=1)
            nc.vector.tensor_copy(st_sb[:], st_ps[:])
            st_in = dr.tile([1, 512], F32, name=f"stin{layer}")
            st_out = dr.tile([1, 512], F32, name=f"stout{layer}", addr_space="Shared")
            nc.gpsimd.dma_start(st_in[:], st_sb[:])
            nc.gpsimd.collective_compute(
                "AllReduce", AX.add, replica_groups=rg,
                ins=[st_in.opt()], outs=[st_out.opt()])
            st2 = sb.tile([1, 512], F32, name=f"st2{layer}", tag="stsb", bufs=1)
            nc.gpsimd.dma_start(st2[:], st_out[:])

            gi = 0 if layer == 0 else 1
            ab = sb.tile([1, 512], F32, name=f"ab{layer}", tag="ab", bufs=1)   # A | B
            mu = sb.tile([1, 256], F32, name=f"mu{layer}", tag="mu", bufs=1)
            nc.vector.tensor_scalar_mul(mu[:], st2[:, 0:256], 1.0 / N)
            var = sb.tile([1, 256], F32, name=f"var{layer}", tag="var", bufs=1)
            nc.vector.tensor_scalar_mul(var[:], st2[:, 256:512], 1.0 / N)
            mu2 = sb.tile([1, 256], F32, name=f"mu2{layer}", tag="mu2", bufs=1)
            nc.vector.tensor_tensor(out=mu2[:], in0=mu[:], in1=mu[:], op=AX.mult)
            nc.vector.tensor_sub(var[:], var[:], mu2[:])
            nc.vector.tensor_scalar_add(var[:], var[:], EPS)
            # rsqrt = exp(-0.5 * ln(var))  (stays in the ln/exp ACT table set)
            lnv = sb.tile([1, 256], F32, name=f"lnv{layer}", tag="lnv", bufs=1)
            nc.scalar.activation(lnv[:], var[:], AF.Ln)
            rs = sb.tile([1, 256], F32, name=f"rs{layer}", tag="rs", bufs=1)
            nc.scalar.activation(rs[:], lnv[:], AF.Exp, bias=0.0, scale=-0.5)
            nc.vector.tensor_tensor(out=ab[:, 0:256], in0=rs[:], in1=bng_s[gi][:], op=AX.mult)
            nc.vector.tensor_tensor(out=mu2[:], in0=mu[:], in1=ab[:, 0:256], op=AX.mult)
            nc.vector.tensor_tensor(out=ab[:, 256:512], in0=bnb_s[gi][:], in1=mu2[:], op=AX.subtract)
            ab_bc = sb.tile([128, 512], F32, name=f"abbc{layer}", tag="abbc", bufs=1)
            nc.gpsimd.partition_broadcast(ab_bc[:], ab[:])

            h_dram = dr.tile([SHP, 256], BF, name=f"h{layer}")
            for b in range(NT):
                y = res.tile([128, 256], BF, name=f"h{layer}_{b}", tag=f"h{layer % 2}_{b}")
                nc.vector.tensor_tensor(out=y[:], in0=usq[b][:], in1=ab_bc[:, 0:256], op=AX.mult)
                nc.vector.tensor_add(y[:], y[:], ab_bc[:, 256:512])
                if elu:
                    ymin = sb.tile([128, 256], BF, name=f"ymin{layer}_{b}", tag="ymin")
                    nc.vector.tensor_scalar_min(ymin[:], y[:], 0.0)
                    expn = sb.tile([128, 256], BF, name=f"expn{layer}_{b}", tag="expn")
                    nc.scalar.activation(expn[:], ymin[:], AF.Exp)
                    nc.scalar.activation(y[:], y[:], AF.Relu)
                    nc.vector.tensor_add(y[:], y[:], expn[:])
                    nc.vector.tensor_scalar_add(y[:], y[:], -1.0)
                h_sb[b] = y
                nc.sync.dma_start(h_dram[b * 128:(b + 1) * 128, :], y[:])
            return h_dram

        # ---------------- main network ----------------
        h_dram = h0_dram
        kdim = NFEAT
        for layer in range(NLAYERS):
            xl_full, xr_tb = node_transform(layer, h_dram, kdim)
            usq, st_ps = edge_phase(layer, xl_full, xr_tb)
            h_dram = bn_tail(layer, usq, st_ps, elu=(layer > 0))
            kdim = 256

        # ---------------- head ----------------
        for nt in range(NT):
            r0 = nt * 128
            hT = []
            for k in range(2):
                t = sb.tile([128, 128], BF, name=f"hTh_{nt}_{k}", tag=f"hT{k}")
                nc.sync.dma_start(out=t[:], in_=h_dram[r0:r0 + 128, k * 128:(k + 1) * 128], transpose=True)
                hT.append(t)
            y1ps = ps.tile([128, 32], F32, name=f"y1ps{nt}", tag="out")
            for k in range(2):
                nc.tensor.matmul(out=y1ps[:], lhsT=hT[k][:], rhs=lin0w_s[:, k * 32:(k + 1) * 32],
                                 start=(k == 0), stop=False)
            nc.tensor.matmul(out=y1ps[:], lhsT=onesr_s[:], rhs=lin0b_s[:], start=False, stop=True)
            y1 = sb.tile([128, 32], BF, name=f"y1_{nt}", tag="y1")
            nc.scalar.copy(y1[:], y1ps[:])
            ymin = sb.tile([128, 32], BF, name=f"hymin{nt}", tag="hymin")
            nc.vector.tensor_scalar_min(ymin[:], y1[:], 0.0)
            expn = sb.tile([128, 32], BF, name=f"hexpn{nt}", tag="hexpn")
            nc.scalar.activation(expn[:], ymin[:], AF.Exp)
            y1e = sb.tile([128, 32], BF, name=f"y1e_{nt}", tag="y1e")
            nc.scalar.activation(y1e[:], y1[:], AF.Relu)
            nc.vector.tensor_add(y1e[:], y1e[:], expn[:])
            nc.vector.tensor_scalar_add(y1e[:], y1e[:], -1.0)
            y1T_ps = ps.tile([32, 128], BF, name=f"y1Tps{nt}", tag="out")
            nc.tensor.transpose(out=y1T_ps[:], in_=y1e[:], identity=ident_s[:])
            y1T = sb.tile([32, 128], BF, name=f"y1T_{nt}", tag="y1T")
            nc.vector.tensor_copy(y1T[:], y1T_ps[:])
            y2ps = ps.tile([128, 10], F32, name=f"y2ps{nt}", tag="out")
            nc.tensor.matmul(out=y2ps[:], lhsT=y1T[:], rhs=lin1w_s[:], start=True, stop=False)
            nc.tensor.matmul(out=y2ps[:], lhsT=onesr_s[:], rhs=lin1b_s[:], start=False, stop=True)
            outf = sb.tile([128, 10], F32, name=f"outf{nt}", tag="outf")
            nc.scalar.copy(outf[:], y2ps[:])
            nc.sync.dma_start(out_d[r0:r0 + 128, :], outf[:])

    nc.finalize()
    return nc


# ----------------------------------------------------------------------------
# host wrapper
# ----------------------------------------------------------------------------

_CACHE = {}


def _prep(inputs):
    x = np.asarray(inputs["x"], np.float32)
    ei = np.asarray(inputs["edge_index"])
    T, off, TC, srcw, dstw, dstl = _preprocess_edges(ei)

    f = lambda k: np.asarray(inputs[k], np.float32)

    # BN0 on the host (depends only on inputs)
    mu = x.mean(0, dtype=np.float64)
    var = ((x.astype(np.float64) - mu) ** 2).mean(0)
    h0 = ((x - mu.astype(np.float32)) / np.sqrt(var + EPS).astype(np.float32)
          * f("norm0_g") + f("norm0_b")).astype(np.float32)

    def pack_w(w):  # [256, 256] -> [128, 512]
        return np.concatenate([w[:128], w[128:]], axis=1)

    wl = [f("conv0_wl")] + [pack_w(f("convs_wl")[i]) for i in range(NCONVS)]
    wr = [f("conv0_wr")] + [pack_w(f("convs_wr")[i]) for i in range(NCONVS)]
    biasr = [(f("conv0_bl") + f("conv0_br"))[None]] + \
            [(f("convs_bl")[i] + f("convs_br")[i])[None] for i in range(NCONVS)]
    blp = [(f("conv0_bl") + f("conv0_bias"))[None]] + \
          [(f("convs_bl")[i] + f("convs_bias")[i])[None] for i in range(NCONVS)]
    att = [f("conv0_att").reshape(1, 256)] + \
          [f("convs_att")[i].reshape(1, 256) for i in range(NCONVS)]

    bf = lambda a: np.ascontiguousarray(a, np.float32).astype(BF16)
    com = {}
    for l in range(NLAYERS):
        com[f"wl{l}"] = bf(wl[l])
        com[f"wr{l}"] = bf(wr[l])
        com[f"biasr{l}"] = bf(biasr[l])
        com[f"blp{l}"] = bf(np.broadcast_to(blp[l], (128, 256)))
        com[f"att{l}"] = bf(np.broadcast_to(att[l], (128, 256)))
    com["bng0"], com["bnb0"] = f("norm1_g")[None].copy(), f("norm1_b")[None].copy()
    com["bng1"], com["bnb1"] = f("norm2_g")[None].copy(), f("norm2_b")[None].copy()
    com["iota"] = bf(np.broadcast_to(np.arange(128, dtype=np.float32)[None], (128, 128)))
    com["ident"] = bf(np.eye(128, dtype=np.float32))
    com["onesc"] = bf(np.ones((128, 1), np.float32))
    com["onesr"] = bf(np.ones((1, 128), np.float32))
    com["lin0w"] = bf(np.concatenate([f("lin0_w")[:128], f("lin0_w")[128:]], axis=1))
    com["lin0b"] = bf(f("lin0_b")[None])
    com["lin1w"] = bf(f("lin1_w"))
    com["lin1b"] = bf(f("lin1_b")[None])

    in_maps = []
    for c in range(NCORES):
        h0c = np.zeros((SHP, NFEAT), np.float32)
        h0c[:SH] = h0[c * SH:(c + 1) * SH]
        m = dict(com)
        m["h0s"] = bf(h0c)
        m["srcw"] = srcw[c]
        m["dstw"] = dstw[c]
        m["dstl"] = dstl[c].astype(BF16)
        in_maps.append(m)
    return (tuple(T), tuple(off), TC), in_maps


def run(inputs, trace=False, trace_kwargs=None, **extra):
    key, in_maps = _prep(inputs)
    if key not in _CACHE:
        _CACHE[key] = _build_program(list(key[0]), list(key[1]), key[2])
    nc = _CACHE[key]
    kw = dict(extra)
    if trace:
        kw.update(trace=True, trace_kwargs=trace_kwargs or {})
    r = run_bass_kernel_spmd(nc, in_maps, list(range(NCORES)), **kw)
    out = np.empty((N, NCLASS), np.float32)
    for c in range(NCORES):
        out[c * SH:(c + 1) * SH] = r.results[c]["out"][:SH]
    return out, r


def kernel(**inputs) -> np.ndarray:
    out, _ = run(inputs, trace=False)
    return out


# revision 20
# speedup vs baseline: 1.4870x; 1.0858x over previous
"""Trainium2 Bass kernel for nn_GAT_skip_forward_15135464751860.

4-layer GATv2 + BN + residual + ELU + linear head on a fixed random graph
(N=20000 nodes, E=480000 edges + self loops), sharded over 8 NeuronCores by
destination node.

Strategy per layer (per core, nodes sharded 2500/core, dst-sorted edges):
  P1  node transform: xl = h @ wl (raw), xr~ = h @ wr + (bl+br) via PE;
      xl shards AllGathered into a full-table in HBM for gathers.
  P2  edge phase per 128-dst block: batched dma_gather of xl[src] and
      xr~[dst] (bf16 rows), z = xl+xr~ on GpSimd, leaky via Prelu on ACT,
      att-dot on DVE (mult + strided tree-reduce), p = exp(e) on ACT,
      wp = [z*p | p], one-hot S_T built by is_equal(dstl, iota), and
      out[i,:] = sum_e S[i,e] wp[e,:] accumulated on the PE into PSUM.
      Softmax denominator rides along as wp column block 256:264.
      out = num/s - xr~ + (bl + conv_bias) (+ residual), since
      sum_e a_e * xl[src_e] = sum_e a_e * z_e - xr~[dst].
  BN  stats via ones-matmul into PSUM, AllReduce, normalize + (ELU).
Head: two small matmuls with a PE transpose in between.
"""
import os, sys

os.environ.setdefault("JAX_PLATFORMS", "")
if "/opt/trn_rl_repo" not in sys.path:
    sys.path.insert(0, "/opt/trn_rl_repo")

import numpy as np
import ml_dtypes
from contextlib import ExitStack

import concourse.bass as bass
import concourse.tile as tile
from concourse import bacc, mybir
from concourse.bass_utils import run_bass_kernel_spmd

BF16 = ml_dtypes.bfloat16

# problem dims
N = 20000
E = 480000
NFEAT = 128
H = 8
C = 32
HC = 256
NCLASS = 10
NCONVS = 3
EPS = 1e-5
SLOPE = 0.2

NCORES = 8
SH = N // NCORES          # 2500 real nodes per core
NT = 20                   # dst blocks / node tiles per core (20*128 = 2560)
SHP = NT * 128            # padded shard rows
NLAYERS = 4

F32 = mybir.dt.float32
BF = mybir.dt.bfloat16
I16 = mybir.dt.int16
AX = mybir.AluOpType
AF = mybir.ActivationFunctionType


# ----------------------------------------------------------------------------
# host-side graph preprocessing
# ----------------------------------------------------------------------------

def _preprocess_edges(edge_index):
    src = np.concatenate([np.asarray(edge_index[0], np.int64), np.arange(N)])
    dst = np.concatenate([np.asarray(edge_index[1], np.int64), np.arange(N)])

    core = dst // SH
    per_core = []
    for c in range(NCORES):
        m = core == c
        s, d = src[m], dst[m]
        o = np.argsort(d, kind="stable")
        s, d = s[o], d[o]
        dloc = d - c * SH
        blk = dloc // 128
        per_core.append((s, dloc, blk))

    # uniform tile count per block across cores (SPMD: one program).
    # need[b] = pad-dst slots that must receive a dummy edge so their softmax
    # denominator stays finite (0 x inf -> NaN otherwise).
    need = np.array([128 - min(128, SH - b * 128) for b in range(NT)], np.int64)
    T = np.zeros(NT, np.int64)
    for c in range(NCORES):
        _, _, blk = per_core[c]
        cnt = np.bincount(blk, minlength=NT)
        T = np.maximum(T, (cnt + need + 127) // 128)
    T = np.maximum(T, 1)
    off = np.zeros(NT + 1, np.int64)
    off[1:] = np.cumsum(T)
    TC = int(off[-1])

    srcw = np.zeros((NCORES, 128, TC * 8), np.int16)
    dstw = np.zeros((NCORES, 128, TC * 8), np.int16)
    dstl = np.full((NCORES, 128, TC), -1.0, np.float32)

    for c in range(NCORES):
        s, dloc, blk = per_core[c]
        for b in range(NT):
            m = blk == b
            sb_, db_ = s[m], dloc[m]
            L = int(T[b]) * 128
            srow = np.zeros(L, np.int64)
            drow = np.zeros(L, np.int64)
            dl = np.full(L, -1.0, np.float32)
            n = len(sb_)
            srow[:n] = (sb_ // SH) * SHP + (sb_ % SH)   # row in AllGathered table
            drow[:n] = db_                              # row in local xr table
            dl[:n] = db_ - b * 128                      # in [0,128)
            nreal = min(128, SH - b * 128)
            for k in range(128 - nreal):                # dummy edges -> pad dsts
                dl[n + k] = nreal + k
            # wrapped int16 idx layout for dma_gather: arr[p, s] = idx[s*16+p],
            # replicated over the 8 Q7 cores (16 partitions * 8)
            sw = srow.reshape(-1, 16).T.astype(np.int16)
            dw = drow.reshape(-1, 16).T.astype(np.int16)
            c0 = int(off[b])
            srcw[c, :, c0 * 8:(c0 + int(T[b])) * 8] = np.tile(sw, (8, 1))
            dstw[c, :, c0 * 8:(c0 + int(T[b])) * 8] = np.tile(dw, (8, 1))
            dstl[c, :, c0:c0 + int(T[b])] = dl.reshape(int(T[b]), 128).T

    return [int(t) for t in T], [int(o) for o in off], TC, srcw, dstw, dstl


# ----------------------------------------------------------------------------
# device program
# ----------------------------------------------------------------------------

def _build_program(T, off, TC):
    TMAX = max(T)
    nc = bacc.Bacc(None, target_bir_lowering=False, num_devices=NCORES)

    # --- external inputs -----------------------------------------------------
    h0s_d = nc.dram_tensor("h0s", [SHP, NFEAT], BF, kind="ExternalInput")
    srcw_d = nc.dram_tensor("srcw", [128, TC * 8], I16, kind="ExternalInput")
    dstw_d = nc.dram_tensor("dstw", [128, TC * 8], I16, kind="ExternalInput")
    dstl_d = nc.dram_tensor("dstl", [128, TC], BF, kind="ExternalInput")

    wl_d = [nc.dram_tensor(f"wl{l}", [128, 256 if l == 0 else 512], BF, kind="ExternalInput") for l in range(NLAYERS)]
    wr_d = [nc.dram_tensor(f"wr{l}", [128, 256 if l == 0 else 512], BF, kind="ExternalInput") for l in range(NLAYERS)]
    biasr_d = [nc.dram_tensor(f"biasr{l}", [1, 256], BF, kind="ExternalInput") for l in range(NLAYERS)]
    blp_d = [nc.dram_tensor(f"blp{l}", [128, 256], BF, kind="ExternalInput") for l in range(NLAYERS)]
    att_d = [nc.dram_tensor(f"att{l}", [128, 256], BF, kind="ExternalInput") for l in range(NLAYERS)]
    bng_d = [nc.dram_tensor(f"bng{s}", [1, 256], F32, kind="ExternalInput") for s in range(2)]
    bnb_d = [nc.dram_tensor(f"bnb{s}", [1, 256], F32, kind="ExternalInput") for s in range(2)]
    iota_d = nc.dram_tensor("iota", [128, 128], BF, kind="ExternalInput")
    ident_d = nc.dram_tensor("ident", [128, 128], BF, kind="ExternalInput")
    onesc_d = nc.dram_tensor("onesc", [128, 1], BF, kind="ExternalInput")
    onesr_d = nc.dram_tensor("onesr", [1, 128], BF, kind="ExternalInput")
    lin0w_d = nc.dram_tensor("lin0w", [128, 2 * 32], BF, kind="ExternalInput")
    lin0b_d = nc.dram_tensor("lin0b", [1, 32], BF, kind="ExternalInput")
    lin1w_d = nc.dram_tensor("lin1w", [32, 10], BF, kind="ExternalInput")
    lin1b_d = nc.dram_tensor("lin1b", [1, 10], BF, kind="ExternalInput")

    out_d = nc.dram_tensor("out", [SHP, NCLASS], F32, kind="ExternalOutput")

    rg = [list(range(NCORES))]

    with tile.TileContext(nc) as tc, ExitStack() as ctx:
        cst = ctx.enter_context(tc.tile_pool(name="cst", bufs=1))
        sb = ctx.enter_context(tc.tile_pool(name="sb", bufs=2))
        big = ctx.enter_context(tc.tile_pool(name="big", bufs=2))
        res = ctx.enter_context(tc.tile_pool(name="res", bufs=1))
        ps = ctx.enter_context(tc.tile_pool(name="ps", bufs=2, space="PSUM"))
        pstat = ctx.enter_context(tc.tile_pool(name="pstat", bufs=1, space="PSUM"))
        pw = ctx.enter_context(tc.tile_pool(name="pw", bufs=2, space="PSUM"))
        pz = ctx.enter_context(tc.tile_pool(name="pz", bufs=2, space="PSUM"))
        psT = ctx.enter_context(tc.tile_pool(name="psT", bufs=1, space="PSUM"))
        # 2+1+2+2+1 = 8 PSUM banks
        dr = ctx.enter_context(tc.tile_pool(name="dr", bufs=1, space="DRAM"))

        # --- load constants --------------------------------------------------
        def cload(dram, shape, dtype, name):
            t = cst.tile(shape, dtype, name=name)
            nc.sync.dma_start(t[:], dram[:])
            return t

        dstl_s = cload(dstl_d, [128, TC], BF, "dstl_s")
        wl_s = [cload(wl_d[l], [128, 256 if l == 0 else 512], BF, f"wl_s{l}") for l in range(NLAYERS)]
        wr_s = [cload(wr_d[l], [128, 256 if l == 0 else 512], BF, f"wr_s{l}") for l in range(NLAYERS)]
        biasr_s = [cload(biasr_d[l], [1, 256], BF, f"biasr_s{l}") for l in range(NLAYERS)]
        blp_s = [cload(blp_d[l], [128, 256], BF, f"blp_s{l}") for l in range(NLAYERS)]
        att_s = [cload(att_d[l], [128, 256], BF, f"att_s{l}") for l in range(NLAYERS)]
        bng_s = [cload(bng_d[s], [1, 256], F32, f"bng_s{s}") for s in range(2)]
        bnb_s = [cload(bnb_d[s], [1, 256], F32, f"bnb_s{s}") for s in range(2)]
        iota_s = cload(iota_d, [128, 128], BF, "iota_s")
        ident_s = cload(ident_d, [128, 128], BF, "ident_s")
        onesc_s = cload(onesc_d, [128, 1], BF, "onesc_s")
        onesr_s = cload(onesr_d, [1, 128], BF, "onesr_s")
        lin0w_s = cload(lin0w_d, [128, 64], BF, "lin0w_s")
        lin0b_s = cload(lin0b_d, [1, 32], BF, "lin0b_s")
        lin1w_s = cload(lin1w_d, [32, 10], BF, "lin1w_s")
        lin1b_s = cload(lin1b_d, [1, 10], BF, "lin1b_s")
        alpha_s = cst.tile([128, 1], F32, name="alpha_s")
        nc.vector.memset(alpha_s[:], SLOPE)

        h0_dram = h0s_d          # layer-0 node features (bf16, [SHP, 128])
        h_sb = [None] * NT       # SBUF resident h tiles (residual input)

        def node_transform(layer, h_dram, kdim):
            """xl/xr tables for this layer from h_dram [SHP, kdim]."""
            xl_sh = dr.tile([SHP, 256], BF, name=f"xl_sh{layer}")
            xr_tb = dr.tile([SHP, 256], BF, name=f"xr_tb{layer}")
            nhalf = kdim // 128
            for nt in range(NT):
                r0 = nt * 128
                hT = []
                for k in range(nhalf):
                    t = sb.tile([128, 128], BF, name=f"hT{layer}_{nt}_{k}", tag=f"hT{k}")
                    eng = nc.sync if (nt % 2 == 0) else nc.scalar
                    eng.dma_start(out=t[:], in_=h_dram[r0:r0 + 128, k * 128:(k + 1) * 128], transpose=True)
                    hT.append(t)
                xl_ps = pw.tile([128, 256], F32, name=f"xlps{layer}_{nt}", tag="xlps")
                for k in range(nhalf):
                    nc.tensor.matmul(out=xl_ps[:], lhsT=hT[k][:], rhs=wl_s[layer][:, k * 256:(k + 1) * 256],
                                     start=(k == 0), stop=(k == nhalf - 1))
                xl_sb = sb.tile([128, 256], BF, name=f"xlsb{layer}_{nt}", tag="xlsb")
                nc.scalar.copy(xl_sb[:], xl_ps[:])
                nc.sync.dma_start(xl_sh[r0:r0 + 128, :], xl_sb[:])

                xr_ps = pw.tile([128, 256], F32, name=f"xrps{layer}_{nt}", tag="xlps")
                for k in range(nhalf):
                    nc.tensor.matmul(out=xr_ps[:], lhsT=hT[k][:], rhs=wr_s[layer][:, k * 256:(k + 1) * 256],
                                     start=(k == 0), stop=False)
                nc.tensor.matmul(out=xr_ps[:], lhsT=onesr_s[:], rhs=biasr_s[layer][:],
                                 start=False, stop=True)
                xr_sb = sb.tile([128, 256], BF, name=f"xrsb{layer}_{nt}", tag="xrsb")
                nc.scalar.copy(xr_sb[:], xr_ps[:])
                nc.sync.dma_start(xr_tb[r0:r0 + 128, :], xr_sb[:])

            xl_full = dr.tile([NCORES * SHP, 256], BF, name=f"xl_full{layer}", addr_space="Shared")
            nc.gpsimd.collective_compute(
                "AllGather", AX.bypass, replica_groups=rg,
                ins=[xl_sh.opt()], outs=[xl_full.opt()])
            return xl_full, xr_tb

        def edge_phase(layer, xl_full, xr_tb, st_dram, s_dram):
            """GATv2 aggregation; returns list of usq tiles ([128,512]: u|u^2)
            and the stats psum tile."""
            st_ps = pstat.tile([1, 512], F32, name=f"stats{layer}", tag="stats")
            # materialize att without a step-0 broadcast (keeps DVE 2x mode)
            attf = cst.tile([128, TMAX, 256], BF, name=f"attf{layer}", tag="attf")
            nc.vector.tensor_copy(
                attf[:], att_s[layer][:, None, :].to_broadcast([128, TMAX, 256]))
            usq = [None] * NT
            for b in range(NT):
                Tb = T[b]
                L = Tb * 128
                c0 = off[b]
                siw = sb.tile([128, Tb * 8], I16, name=f"siw{layer}_{b}", tag="siw", bufs=3)
                nc.sync.dma_start(siw[:], srcw_d[:, c0 * 8:(c0 + Tb) * 8])
                xg = big.tile([128, Tb, 256], BF, name=f"xg{layer}_{b}", tag="z", bufs=3)
                nc.gpsimd.dma_gather(
                    out_ap=xg[:], in_ap=xl_full[:], idxs_ap=siw[:],
                    num_idxs=L, num_idxs_reg=L, elem_size=256, single_packet=False)
                xrblk = sb.tile([128, 256], BF, name=f"xrblk{layer}_{b}", tag="xrblk")
                nc.sync.dma_start(xrblk[:], xr_tb[b * 128:(b + 1) * 128, :])
                ST = big.tile([128, Tb, 128], BF, name=f"ST{layer}_{b}", tag="ST")
                Sblk = big.tile([128, Tb, 128], BF, name=f"Sblk{layer}_{b}", tag="Sblk")
                if layer == 0:
                    # build one-hots once: S_T[e, i] = (dstl[e] == i); S = transpose
                    nc.vector.tensor_tensor(
                        out=ST[:],
                        in0=dstl_s[:, c0:c0 + Tb, None].to_broadcast([128, Tb, 128]),
                        in1=iota_s[:, None, :].to_broadcast([128, Tb, 128]),
                        op=AX.is_equal)
                    for j in range(Tb):
                        sT_ps = psT.tile([128, 128], BF, name=f"sT{layer}_{b}_{j}", tag="sT")
                        nc.tensor.transpose(out=sT_ps[:], in_=ST[:, j, :], identity=ident_s[:])
                        if j % 2 == 0:
                            nc.vector.tensor_copy(Sblk[:, j, :], sT_ps[:])
                        else:
                            nc.scalar.copy(Sblk[:, j, :], sT_ps[:])
                    nc.sync.dma_start(st_dram[:, c0 * 128:(c0 + Tb) * 128], ST[:].rearrange("p t i -> p (t i)"))
                    nc.scalar.dma_start(s_dram[:, c0 * 128:(c0 + Tb) * 128], Sblk[:].rearrange("p t i -> p (t i)"))
                else:
                    nc.sync.dma_start(ST[:].rearrange("p t i -> p (t i)"), st_dram[:, c0 * 128:(c0 + Tb) * 128])
                    nc.scalar.dma_start(Sblk[:].rearrange("p t i -> p (t i)"), s_dram[:, c0 * 128:(c0 + Tb) * 128])
                # per tile: z = S.T @ xr_block + I @ xl_g  on the PE (no xr gather)
                t = big.tile([128, Tb, 256], BF, name=f"t{layer}_{b}", tag="t", bufs=1)
                for j in range(Tb):
                    z_ps = pz.tile([128, 256], F32, name=f"zps{layer}_{b}_{j}", tag="zps")
                    nc.tensor.matmul(out=z_ps[:], lhsT=Sblk[:, j, :], rhs=xrblk[:], start=True, stop=False)
                    nc.tensor.matmul(out=z_ps[:], lhsT=ident_s[:], rhs=xg[:, j, :], start=False, stop=True)
                    # leaky(z) straight out of PSUM; z itself is never needed again:
                    # sum_e a_e xl[src_e] = num/s because the xr part cancels (sum a = 1)
                    nc.scalar.activation(t[:, j, :], z_ps[:], AF.Prelu, bias=0.0, scale=1.0, alpha=alpha_s[:])
                nc.vector.tensor_tensor(out=t[:], in0=t[:], in1=attf[:, :Tb, :], op=AX.mult)
                # e = per-head sum of t  (strided tree reduce)
                t4 = t[:].rearrange("p t (h c) -> p t h c", c=32)
                r16 = big.tile([128, Tb, 8, 16], BF, name=f"r16_{layer}_{b}", tag="r16", bufs=1)
                nc.vector.tensor_tensor(out=r16[:], in0=t4[:, :, :, 0:16], in1=t4[:, :, :, 16:32], op=AX.add)
                r8 = big.tile([128, Tb, 8, 8], BF, name=f"r8_{layer}_{b}", tag="r8", bufs=1)
                nc.vector.tensor_tensor(out=r8[:], in0=r16[:, :, :, 0:8], in1=r16[:, :, :, 8:16], op=AX.add)
                r4 = big.tile([128, Tb, 8, 4], BF, name=f"r4_{layer}_{b}", tag="r4", bufs=1)
                nc.vector.tensor_tensor(out=r4[:], in0=r8[:, :, :, 0:4], in1=r8[:, :, :, 4:8], op=AX.add)
                r2 = big.tile([128, Tb, 8, 2], BF, name=f"r2_{layer}_{b}", tag="r2", bufs=1)
                nc.vector.tensor_tensor(out=r2[:], in0=r4[:, :, :, 0:2], in1=r4[:, :, :, 2:4], op=AX.add)
                e = big.tile([128, Tb, 8], F32, name=f"e{layer}_{b}", tag="e", bufs=1)
                nc.vector.tensor_tensor(out=e[:], in0=r2[:, :, :, 0], in1=r2[:, :, :, 1], op=AX.add)
                # wp = [xl_g * p | p]   (xr part of the numerator cancels with -xr~)
                wp = big.tile([128, Tb, 264], BF, name=f"wp{layer}_{b}", tag="wp")
                p_sb = big.tile([128, Tb, 8], BF, name=f"p{layer}_{b}", tag="p", bufs=2)
                nc.scalar.activation(p_sb[:], e[:], AF.Exp)
                nc.vector.tensor_copy(wp[:, :, 256:264], p_sb[:])
                nc.vector.tensor_tensor(
                    out=wp[:, :, 0:256].rearrange("p t (h c) -> p t h c", c=32),
                    in0=xg[:].rearrange("p t (h c) -> p t h c", c=32),
                    in1=p_sb[:, :, :, None].to_broadcast([128, Tb, 8, 32]),
                    op=AX.mult)
                out_ps = ps.tile([128, 264], F32, name=f"ops{layer}_{b}", tag="out")
                for j in range(Tb):
                    nc.tensor.matmul(out=out_ps[:], lhsT=ST[:, j, :], rhs=wp[:, j, :],
                                     start=(j == 0), stop=(j == Tb - 1))
                # finalize: outn = num/s ; u = outn - xrhat (+ h_res)
                rec = sb.tile([128, 8], F32, name=f"rec{layer}_{b}", tag="rec")
                nc.vector.reciprocal(rec[:], out_ps[:, 256:264])
                us = res.tile([128, 256], BF, name=f"u{layer}_{b}", tag=f"u{b}")
                nc.vector.tensor_tensor(
                    out=us[:].rearrange("p (h c) -> p h c", c=32),
                    in0=out_ps[:, 0:256].rearrange("p (h c) -> p h c", c=32),
                    in1=rec[:, :, None].to_broadcast([128, 8, 32]), op=AX.mult)
                nc.vector.tensor_add(us[:], us[:], blp_s[layer][:])
                if layer > 0:
                    nc.vector.tensor_add(us[:], us[:], h_sb[b][:])
                sq = sb.tile([128, 256], BF, name=f"sq{layer}_{b}", tag="sq")
                nc.scalar.square(sq[:], us[:])
                nreal = 128 if b < NT - 1 else SH - (NT - 1) * 128
                nc.tensor.matmul(out=st_ps[0:1, 0:256], lhsT=onesc_s[0:nreal, :], rhs=us[0:nreal, :],
                                 start=(b == 0), stop=(b == NT - 1), skip_group_check=True)
                nc.tensor.matmul(out=st_ps[0:1, 256:512], lhsT=onesc_s[0:nreal, :], rhs=sq[0:nreal, :],
                                 start=(b == 0), stop=(b == NT - 1), skip_group_check=True)
                usq[b] = us
            return usq, st_ps

        def bn_tail(layer, usq, st_ps, elu):
            """AllReduce stats, normalize (+ELU); returns h dram + fills h_sb."""
            st_sb = sb.tile([1, 512], F32, name=f"stsb{layer}", tag="stsb", bufs=1)
            nc.vector.tensor_copy(st_sb[:], st_ps[:])
            st_in = dr.tile([1, 512], F32, name=f"stin{layer}")
            st_out = dr.tile([1, 512], F32, name=f"stout{layer}", addr_space="Shared")
            nc.gpsimd.dma_start(st_in[:], st_sb[:])
            nc.gpsimd.collective_compute(
                "AllReduce", AX.add, replica_groups=rg,
                ins=[st_in.opt()], outs=[st_out.opt()])
            st2 = sb.tile([1, 512], F32, name=f"st2{layer}", tag="stsb", bufs=1)
            nc.gpsimd.dma_start(st2[:], st_out[:])

            gi = 0 if layer == 0 else 1
            ab = sb.tile([1, 512], F32, name=f"ab{layer}", tag="ab", bufs=1)   # A | B
            mu = sb.tile([1, 256], F32, name=f"mu{layer}", tag="mu", bufs=1)
            nc.vector.tensor_scalar_mul(mu[:], st2[:, 0:256], 1.0 / N)
            var = sb.tile([1, 256], F32, name=f"var{layer}", tag="var", bufs=1)
            nc.vector.tensor_scalar_mul(var[:], st2[:, 256:512], 1.0 / N)
            mu2 = sb.tile([1, 256], F32, name=f"mu2{layer}", tag="mu2", bufs=1)
            nc.vector.tensor_tensor(out=mu2[:], in0=mu[:], in1=mu[:], op=AX.mult)
            nc.vector.tensor_sub(var[:], var[:], mu2[:])
            nc.vector.tensor_scalar_add(var[:], var[:], EPS)
            # rsqrt = exp(-0.5 * ln(var))  (stays in the ln/exp ACT table set)
            lnv = sb.tile([1, 256], F32, name=f"lnv{layer}", tag="lnv", bufs=1)
            nc.scalar.activation(lnv[:], var[:], AF.Ln)
            rs = sb.tile([1, 256], F32, name=f"rs{layer}", tag="rs", bufs=1)
            nc.scalar.activation(rs[:], lnv[:], AF.Exp, bias=0.0, scale=-0.5)
            nc.vector.tensor_tensor(out=ab[:, 0:256], in0=rs[:], in1=bng_s[gi][:], op=AX.mult)
            nc.vector.tensor_tensor(out=mu2[:], in0=mu[:], in1=ab[:, 0:256], op=AX.mult)
            nc.vector.tensor_tensor(out=ab[:, 256:512], in0=bnb_s[gi][:], in1=mu2[:], op=AX.subtract)
            ab_bc = sb.tile([128, 512], F32, name=f"abbc{layer}", tag="abbc", bufs=1)
            nc.gpsimd.partition_broadcast(ab_bc[:], ab[:])

            h_dram = dr.tile([SHP, 256], BF, name=f"h{layer}")
            for b in range(NT):
                y = res.tile([128, 256], BF, name=f"h{layer}_{b}", tag=f"h{layer % 2}_{b}")
                nc.vector.tensor_tensor(out=y[:], in0=usq[b][:], in1=ab_bc[:, 0:256], op=AX.mult)
                nc.vector.tensor_add(y[:], y[:], ab_bc[:, 256:512])
                if elu:
                    ymin = sb.tile([128, 256], BF, name=f"ymin{layer}_{b}", tag="ymin")
                    nc.vector.tensor_scalar_min(ymin[:], y[:], 0.0)
                    expn = sb.tile([128, 256], BF, name=f"expn{layer}_{b}", tag="expn")
                    nc.scalar.activation(expn[:], ymin[:], AF.Exp)
                    nc.scalar.activation(y[:], y[:], AF.Relu)
                    nc.vector.tensor_add(y[:], y[:], expn[:])
                    nc.vector.tensor_scalar_add(y[:], y[:], -1.0)
                h_sb[b] = y
                nc.sync.dma_start(h_dram[b * 128:(b + 1) * 128, :], y[:])
            return h_dram

        # ---------------- main network ----------------
        st_dram = dr.tile([128, TC * 128], BF, name="st_dram")
        s_dram = dr.tile([128, TC * 128], BF, name="s_dram")
        h_dram = h0_dram
        kdim = NFEAT
        for layer in range(NLAYERS):
            xl_full, xr_tb = node_transform(layer, h_dram, kdim)
            usq, st_ps = edge_phase(layer, xl_full, xr_tb, st_dram, s_dram)
            h_dram = bn_tail(layer, usq, st_ps, elu=(layer > 0))
            kdim = 256

        # ---------------- head ----------------
        for nt in range(NT):
            r0 = nt * 128
            hT = []
            for k in range(2):
                t = sb.tile([128, 128], BF, name=f"hTh_{nt}_{k}", tag=f"hT{k}")
                nc.sync.dma_start(out=t[:], in_=h_dram[r0:r0 + 128, k * 128:(k + 1) * 128], transpose=True)
                hT.append(t)
            y1ps = ps.tile([128, 32], F32, name=f"y1ps{nt}", tag="out")
            for k in range(2):
                nc.tensor.matmul(out=y1ps[:], lhsT=hT[k][:], rhs=lin0w_s[:, k * 32:(k + 1) * 32],
                                 start=(k == 0), stop=False)
            nc.tensor.matmul(out=y1ps[:], lhsT=onesr_s[:], rhs=lin0b_s[:], start=False, stop=True)
            y1 = sb.tile([128, 32], BF, name=f"y1_{nt}", tag="y1")
            nc.scalar.copy(y1[:], y1ps[:])
            ymin = sb.tile([128, 32], BF, name=f"hymin{nt}", tag="hymin")
            nc.vector.tensor_scalar_min(ymin[:], y1[:], 0.0)
            expn = sb.tile([128, 32], BF, name=f"hexpn{nt}", tag="hexpn")
            nc.scalar.activation(expn[:], ymin[:], AF.Exp)
            y1e = sb.tile([128, 32], BF, name=f"y1e_{nt}", tag="y1e")
            nc.scalar.activation(y1e[:], y1[:], AF.Relu)
            nc.vector.tensor_add(y1e[:], y1e[:], expn[:])
            nc.vector.tensor_scalar_add(y1e[:], y1e[:], -1.0)
            y1T_ps = ps.tile([32, 128], BF, name=f"y1Tps{nt}", tag="out")
            nc.tensor.transpose(out=y1T_ps[:], in_=y1e[:], identity=ident_s[:])
            y1T = sb.tile([32, 128], BF, name=f"y1T_{nt}", tag="y1T")
            nc.vector.tensor_copy(y1T[:], y1T_ps[:])
            y2ps = ps.tile([128, 10], F32, name=f"y2ps{nt}", tag="out")
            nc.tensor.matmul(out=y2ps[:], lhsT=y1T[:], rhs=lin1w_s[:], start=True, stop=False)
            nc.tensor.matmul(out=y2ps[:], lhsT=onesr_s[:], rhs=lin1b_s[:], start=False, stop=True)
            outf = sb.tile([128, 10], F32, name=f"outf{nt}", tag="outf")
            nc.scalar.copy(outf[:], y2ps[:])
            nc.sync.dma_start(out_d[r0:r0 + 128, :], outf[:])

    nc.finalize()
    return nc


# ----------------------------------------------------------------------------
# host wrapper
# ----------------------------------------------------------------------------

_CACHE = {}


def _prep(inputs):
    x = np.asarray(inputs["x"], np.float32)
    ei = np.asarray(inputs["edge_index"])
    T, off, TC, srcw, dstw, dstl = _preprocess_edges(ei)

    f = lambda k: np.asarray(inputs[k], np.float32)

    # BN0 on the host (depends only on inputs)
    mu = x.mean(0, dtype=np.float64)
    var = ((x.astype(np.float64) - mu) ** 2).mean(0)
    h0 = ((x - mu.astype(np.float32)) / np.sqrt(var + EPS).astype(np.float32)
          * f("norm0_g") + f("norm0_b")).astype(np.float32)

    def pack_w(w):  # [256, 256] -> [128, 512]
        return np.concatenate([w[:128], w[128:]], axis=1)

    wl = [f("conv0_wl")] + [pack_w(f("convs_wl")[i]) for i in range(NCONVS)]
    wr = [f("conv0_wr")] + [pack_w(f("convs_wr")[i]) for i in range(NCONVS)]
    biasr = [(f("conv0_bl") + f("conv0_br"))[None]] + \
            [(f("convs_bl")[i] + f("convs_br")[i])[None] for i in range(NCONVS)]
    blp = [(f("conv0_bl") + f("conv0_bias"))[None]] + \
          [(f("convs_bl")[i] + f("convs_bias")[i])[None] for i in range(NCONVS)]
    att = [f("conv0_att").reshape(1, 256)] + \
          [f("convs_att")[i].reshape(1, 256) for i in range(NCONVS)]

    bf = lambda a: np.ascontiguousarray(a, np.float32).astype(BF16)
    com = {}
    for l in range(NLAYERS):
        com[f"wl{l}"] = bf(wl[l])
        com[f"wr{l}"] = bf(wr[l])
        com[f"biasr{l}"] = bf(biasr[l])
        com[f"blp{l}"] = bf(np.broadcast_to(blp[l], (128, 256)))
        com[f"att{l}"] = bf(np.broadcast_to(att[l], (128, 256)))
    com["bng0"], com["bnb0"] = f("norm1_g")[None].copy(), f("norm1_b")[None].copy()
    com["bng1"], com["bnb1"] = f("norm2_g")[None].copy(), f("norm2_b")[None].copy()
    com["iota"] = bf(np.broadcast_to(np.arange(128, dtype=np.float32)[None], (128, 128)))
    com["ident"] = bf(np.eye(128, dtype=np.float32))
    com["onesc"] = bf(np.ones((128, 1), np.float32))
    com["onesr"] = bf(np.ones((1, 128), np.float32))
    com["lin0w"] = bf(np.concatenate([f("lin0_w")[:128], f("lin0_w")[128:]], axis=1))
    com["lin0b"] = bf(f("lin0_b")[None])
    com["lin1w"] = bf(f("lin1_w"))
    com["lin1b"] = bf(f("lin1_b")[None])

    in_maps = []
    for c in range(NCORES):
        h0c = np.zeros((SHP, NFEAT), np.float32)
        h0c[:SH] = h0[c * SH:(c + 1) * SH]
        m = dict(com)
        m["h0s"] = bf(h0c)
        m["srcw"] = srcw[c]
        m["dstw"] = dstw[c]
        m["dstl"] = dstl[c].astype(BF16)
        in_maps.append(m)
    return (tuple(T), tuple(off), TC), in_maps


def run(inputs, trace=False, trace_kwargs=None, **extra):
    key, in_maps = _prep(inputs)
    if key not in _CACHE:
        _CACHE[key] = _build_program(list(key[0]), list(key[1]), key[2])
    nc = _CACHE[key]
    kw = dict(extra)
    if trace:
        kw.update(trace=True, trace_kwargs=trace_kwargs or {})
    r = run_bass_kernel_spmd(nc, in_maps, list(range(NCORES)), **kw)
    out = np.empty((N, NCLASS), np.float32)
    for c in range(NCORES):
        out[c * SH:(c + 1) * SH] = r.results[c]["out"][:SH]
    return out, r


def kernel(**inputs) -> np.ndarray:
    out, _ = run(inputs, trace=False)
    return out


# revision 21
# speedup vs baseline: 1.5926x; 1.0710x over previous
"""Trainium2 Bass kernel for nn_GAT_skip_forward_15135464751860.

4-layer GATv2 + BN + residual + ELU + linear head on a fixed random graph
(N=20000 nodes, E=480000 edges + self loops), sharded over 8 NeuronCores by
destination node.

Strategy per layer (per core, nodes sharded 2500/core, dst-sorted edges):
  P1  node transform: xl = h @ wl (raw), xr~ = h @ wr + (bl+br) via PE;
      xl shards AllGathered into a full-table in HBM for gathers.
  P2  edge phase per 128-dst block: batched dma_gather of xl[src] and
      xr~[dst] (bf16 rows), z = xl+xr~ on GpSimd, leaky via Prelu on ACT,
      att-dot on DVE (mult + strided tree-reduce), p = exp(e) on ACT,
      wp = [z*p | p], one-hot S_T built by is_equal(dstl, iota), and
      out[i,:] = sum_e S[i,e] wp[e,:] accumulated on the PE into PSUM.
      Softmax denominator rides along as wp column block 256:264.
      out = num/s - xr~ + (bl + conv_bias) (+ residual), since
      sum_e a_e * xl[src_e] = sum_e a_e * z_e - xr~[dst].
  BN  stats via ones-matmul into PSUM, AllReduce, normalize + (ELU).
Head: two small matmuls with a PE transpose in between.
"""
import os, sys

os.environ.setdefault("JAX_PLATFORMS", "")
if "/opt/trn_rl_repo" not in sys.path:
    sys.path.insert(0, "/opt/trn_rl_repo")

import numpy as np
import ml_dtypes
from contextlib import ExitStack

import concourse.bass as bass
import concourse.tile as tile
from concourse import bacc, mybir
from concourse.bass_utils import run_bass_kernel_spmd

BF16 = ml_dtypes.bfloat16

# problem dims
N = 20000
E = 480000
NFEAT = 128
H = 8
C = 32
HC = 256
NCLASS = 10
NCONVS = 3
EPS = 1e-5
SLOPE = 0.2

NCORES = 8
SH = N // NCORES          # 2500 real nodes per core
NT = 20                   # dst blocks / node tiles per core (20*128 = 2560)
SHP = NT * 128            # padded shard rows
NLAYERS = 4

F32 = mybir.dt.float32
BF = mybir.dt.bfloat16
I16 = mybir.dt.int16
AX = mybir.AluOpType
AF = mybir.ActivationFunctionType


# ----------------------------------------------------------------------------
# host-side graph preprocessing
# ----------------------------------------------------------------------------

def _preprocess_edges(edge_index):
    src = np.concatenate([np.asarray(edge_index[0], np.int64), np.arange(N)])
    dst = np.concatenate([np.asarray(edge_index[1], np.int64), np.arange(N)])

    core = dst // SH
    per_core = []
    for c in range(NCORES):
        m = core == c
        s, d = src[m], dst[m]
        o = np.argsort(d, kind="stable")
        s, d = s[o], d[o]
        dloc = d - c * SH
        blk = dloc // 128
        per_core.append((s, dloc, blk))

    # uniform tile count per block across cores (SPMD: one program).
    # need[b] = pad-dst slots that must receive a dummy edge so their softmax
    # denominator stays finite (0 x inf -> NaN otherwise).
    need = np.array([128 - min(128, SH - b * 128) for b in range(NT)], np.int64)
    T = np.zeros(NT, np.int64)
    for c in range(NCORES):
        _, _, blk = per_core[c]
        cnt = np.bincount(blk, minlength=NT)
        T = np.maximum(T, (cnt + need + 127) // 128)
    T = np.maximum(T, 1)
    off = np.zeros(NT + 1, np.int64)
    off[1:] = np.cumsum(T)
    TC = int(off[-1])

    srcw = np.zeros((NCORES, 128, TC * 8), np.int16)
    dstw = np.zeros((NCORES, 128, TC * 8), np.int16)
    dstl = np.full((NCORES, 128, TC), -1.0, np.float32)

    for c in range(NCORES):
        s, dloc, blk = per_core[c]
        for b in range(NT):
            m = blk == b
            sb_, db_ = s[m], dloc[m]
            L = int(T[b]) * 128
            srow = np.zeros(L, np.int64)
            drow = np.zeros(L, np.int64)
            dl = np.full(L, -1.0, np.float32)
            n = len(sb_)
            srow[:n] = (sb_ // SH) * SHP + (sb_ % SH)   # row in AllGathered table
            drow[:n] = db_                              # row in local xr table
            dl[:n] = db_ - b * 128                      # in [0,128)
            nreal = min(128, SH - b * 128)
            for k in range(128 - nreal):                # dummy edges -> pad dsts
                dl[n + k] = nreal + k
            # wrapped int16 idx layout for dma_gather: arr[p, s] = idx[s*16+p],
            # replicated over the 8 Q7 cores (16 partitions * 8)
            sw = srow.reshape(-1, 16).T.astype(np.int16)
            dw = drow.reshape(-1, 16).T.astype(np.int16)
            c0 = int(off[b])
            srcw[c, :, c0 * 8:(c0 + int(T[b])) * 8] = np.tile(sw, (8, 1))
            dstw[c, :, c0 * 8:(c0 + int(T[b])) * 8] = np.tile(dw, (8, 1))
            dstl[c, :, c0:c0 + int(T[b])] = dl.reshape(int(T[b]), 128).T

    return [int(t) for t in T], [int(o) for o in off], TC, srcw, dstw, dstl


# ----------------------------------------------------------------------------
# device program
# ----------------------------------------------------------------------------

def _build_program(T, off, TC):
    TMAX = max(T)
    nc = bacc.Bacc(None, target_bir_lowering=False, num_devices=NCORES)

    # --- external inputs -----------------------------------------------------
    h0s_d = nc.dram_tensor("h0s", [SHP, NFEAT], BF, kind="ExternalInput")
    srcw_d = nc.dram_tensor("srcw", [128, TC * 8], I16, kind="ExternalInput")
    dstw_d = nc.dram_tensor("dstw", [128, TC * 8], I16, kind="ExternalInput")
    dstl_d = nc.dram_tensor("dstl", [128, TC], BF, kind="ExternalInput")

    wl_d = [nc.dram_tensor(f"wl{l}", [128, 256 if l == 0 else 512], BF, kind="ExternalInput") for l in range(NLAYERS)]
    wr_d = [nc.dram_tensor(f"wr{l}", [128, 256 if l == 0 else 512], BF, kind="ExternalInput") for l in range(NLAYERS)]
    biasr_d = [nc.dram_tensor(f"biasr{l}", [1, 256], BF, kind="ExternalInput") for l in range(NLAYERS)]
    blp_d = [nc.dram_tensor(f"blp{l}", [128, 256], BF, kind="ExternalInput") for l in range(NLAYERS)]
    att_d = [nc.dram_tensor(f"att{l}", [128, 256], BF, kind="ExternalInput") for l in range(NLAYERS)]
    bng_d = [nc.dram_tensor(f"bng{s}", [1, 256], F32, kind="ExternalInput") for s in range(2)]
    bnb_d = [nc.dram_tensor(f"bnb{s}", [1, 256], F32, kind="ExternalInput") for s in range(2)]
    iota_d = nc.dram_tensor("iota", [128, 128], BF, kind="ExternalInput")
    ident_d = nc.dram_tensor("ident", [128, 128], BF, kind="ExternalInput")
    onesc_d = nc.dram_tensor("onesc", [128, 1], BF, kind="ExternalInput")
    onesr_d = nc.dram_tensor("onesr", [1, 128], BF, kind="ExternalInput")
    lin0w_d = nc.dram_tensor("lin0w", [128, 2 * 32], BF, kind="ExternalInput")
    lin0b_d = nc.dram_tensor("lin0b", [1, 32], BF, kind="ExternalInput")
    lin1w_d = nc.dram_tensor("lin1w", [32, 10], BF, kind="ExternalInput")
    lin1b_d = nc.dram_tensor("lin1b", [1, 10], BF, kind="ExternalInput")

    out_d = nc.dram_tensor("out", [SHP, NCLASS], F32, kind="ExternalOutput")

    rg = [list(range(NCORES))]

    with tile.TileContext(nc) as tc, ExitStack() as ctx:
        cst = ctx.enter_context(tc.tile_pool(name="cst", bufs=1))
        sb = ctx.enter_context(tc.tile_pool(name="sb", bufs=2))
        big = ctx.enter_context(tc.tile_pool(name="big", bufs=2))
        res = ctx.enter_context(tc.tile_pool(name="res", bufs=1))
        ps = ctx.enter_context(tc.tile_pool(name="ps", bufs=2, space="PSUM"))
        pstat = ctx.enter_context(tc.tile_pool(name="pstat", bufs=1, space="PSUM"))
        pz = ctx.enter_context(tc.tile_pool(name="pz", bufs=4, space="PSUM"))
        psT = ctx.enter_context(tc.tile_pool(name="psT", bufs=1, space="PSUM"))
        # ps2 + pstat1 + pz4 + psT1 = 8 PSUM banks
        dr = ctx.enter_context(tc.tile_pool(name="dr", bufs=1, space="DRAM"))

        # --- load constants --------------------------------------------------
        def cload(dram, shape, dtype, name):
            t = cst.tile(shape, dtype, name=name)
            nc.sync.dma_start(t[:], dram[:])
            return t

        dstl_s = cload(dstl_d, [128, TC], BF, "dstl_s")
        wl_s = [cload(wl_d[l], [128, 256 if l == 0 else 512], BF, f"wl_s{l}") for l in range(NLAYERS)]
        wr_s = [cload(wr_d[l], [128, 256 if l == 0 else 512], BF, f"wr_s{l}") for l in range(NLAYERS)]
        biasr_s = [cload(biasr_d[l], [1, 256], BF, f"biasr_s{l}") for l in range(NLAYERS)]
        blp_s = [cload(blp_d[l], [128, 256], BF, f"blp_s{l}") for l in range(NLAYERS)]
        att_s = [cload(att_d[l], [128, 256], BF, f"att_s{l}") for l in range(NLAYERS)]
        bng_s = [cload(bng_d[s], [1, 256], F32, f"bng_s{s}") for s in range(2)]
        bnb_s = [cload(bnb_d[s], [1, 256], F32, f"bnb_s{s}") for s in range(2)]
        iota_s = cload(iota_d, [128, 128], BF, "iota_s")
        ident_s = cload(ident_d, [128, 128], BF, "ident_s")
        onesc_s = cload(onesc_d, [128, 1], BF, "onesc_s")
        onesr_s = cload(onesr_d, [1, 128], BF, "onesr_s")
        lin0w_s = cload(lin0w_d, [128, 64], BF, "lin0w_s")
        lin0b_s = cload(lin0b_d, [1, 32], BF, "lin0b_s")
        lin1w_s = cload(lin1w_d, [32, 10], BF, "lin1w_s")
        lin1b_s = cload(lin1b_d, [1, 10], BF, "lin1b_s")
        alpha_s = cst.tile([128, 1], F32, name="alpha_s")
        nc.vector.memset(alpha_s[:], SLOPE)

        h_sb = [None] * NT       # SBUF resident h tiles (residual input)

        def node_transform(layer, h_dram, kdim):
            """xl/xr tables for this layer from h_dram [SHP, kdim] (layer 0)
            or from resident h_sb tiles (later layers, on-chip transpose)."""
            xl_sh = dr.tile([SHP, 256], BF, name=f"xl_sh{layer}")
            xr_tb = dr.tile([SHP, 256], BF, name=f"xr_tb{layer}")
            nhalf = kdim // 128
            for nt in range(NT):
                r0 = nt * 128
                hT = []
                for k in range(nhalf):
                    t = sb.tile([128, 128], BF, name=f"hT{layer}_{nt}_{k}", tag=f"hT{k}")
                    if layer == 0:
                        eng = nc.sync if (nt % 2 == 0) else nc.scalar
                        eng.dma_start(out=t[:], in_=h_dram[r0:r0 + 128, k * 128:(k + 1) * 128], transpose=True)
                    else:
                        hT_ps = psT.tile([128, 128], BF, name=f"hTp{layer}_{nt}_{k}", tag="sT")
                        nc.tensor.transpose(out=hT_ps[:], in_=h_sb[nt][:, k * 128:(k + 1) * 128],
                                            identity=ident_s[:])
                        if k % 2 == 0:
                            nc.vector.tensor_copy(t[:], hT_ps[:])
                        else:
                            nc.scalar.copy(t[:], hT_ps[:])
                    hT.append(t)
                xl_ps = pz.tile([128, 256], F32, name=f"xlps{layer}_{nt}", tag="zps")
                for k in range(nhalf):
                    nc.tensor.matmul(out=xl_ps[:], lhsT=hT[k][:], rhs=wl_s[layer][:, k * 256:(k + 1) * 256],
                                     start=(k == 0), stop=(k == nhalf - 1))
                xl_sb = sb.tile([128, 256], BF, name=f"xlsb{layer}_{nt}", tag="xlsb")
                nc.scalar.copy(xl_sb[:], xl_ps[:])
                nc.sync.dma_start(xl_sh[r0:r0 + 128, :], xl_sb[:])

                xr_ps = pz.tile([128, 256], F32, name=f"xrps{layer}_{nt}", tag="zps")
                for k in range(nhalf):
                    nc.tensor.matmul(out=xr_ps[:], lhsT=hT[k][:], rhs=wr_s[layer][:, k * 256:(k + 1) * 256],
                                     start=(k == 0), stop=False)
                nc.tensor.matmul(out=xr_ps[:], lhsT=onesr_s[:], rhs=biasr_s[layer][:],
                                 start=False, stop=True)
                xr_sb = sb.tile([128, 256], BF, name=f"xrsb{layer}_{nt}", tag="xrsb")
                nc.scalar.copy(xr_sb[:], xr_ps[:])
                nc.sync.dma_start(xr_tb[r0:r0 + 128, :], xr_sb[:])

            xl_full = dr.tile([NCORES * SHP, 256], BF, name=f"xl_full{layer}", addr_space="Shared")
            nc.gpsimd.collective_compute(
                "AllGather", AX.bypass, replica_groups=rg,
                ins=[xl_sh.opt()], outs=[xl_full.opt()])
            return xl_full, xr_tb

        def edge_phase(layer, xl_full, xr_tb, st_dram, s_dram):
            """GATv2 aggregation; returns list of usq tiles ([128,512]: u|u^2)
            and the stats psum tile."""
            st_ps = pstat.tile([1, 512], F32, name=f"stats{layer}", tag="stats")
            # materialize att without a step-0 broadcast (keeps DVE 2x mode)
            attf = cst.tile([128, TMAX, 256], BF, name=f"attf{layer}", tag="attf")
            nc.vector.tensor_copy(
                attf[:], att_s[layer][:, None, :].to_broadcast([128, TMAX, 256]))
            usq = [None] * NT
            for b in range(NT):
                Tb = T[b]
                L = Tb * 128
                c0 = off[b]
                siw = sb.tile([128, Tb * 8], I16, name=f"siw{layer}_{b}", tag="siw", bufs=3)
                nc.sync.dma_start(siw[:], srcw_d[:, c0 * 8:(c0 + Tb) * 8])
                xg = big.tile([128, Tb, 256], BF, name=f"xg{layer}_{b}", tag="z", bufs=3)
                nc.gpsimd.dma_gather(
                    out_ap=xg[:], in_ap=xl_full[:], idxs_ap=siw[:],
                    num_idxs=L, num_idxs_reg=L, elem_size=256, single_packet=False)
                xrblk = sb.tile([128, 256], BF, name=f"xrblk{layer}_{b}", tag="xrblk")
                nc.sync.dma_start(xrblk[:], xr_tb[b * 128:(b + 1) * 128, :])
                ST = big.tile([128, Tb, 128], BF, name=f"ST{layer}_{b}", tag="ST")
                Sblk = big.tile([128, Tb, 128], BF, name=f"Sblk{layer}_{b}", tag="Sblk")
                if layer == 0:
                    # build one-hots once: S_T[e, i] = (dstl[e] == i); S = transpose
                    nc.vector.tensor_tensor(
                        out=ST[:],
                        in0=dstl_s[:, c0:c0 + Tb, None].to_broadcast([128, Tb, 128]),
                        in1=iota_s[:, None, :].to_broadcast([128, Tb, 128]),
                        op=AX.is_equal)
                    for j in range(Tb):
                        sT_ps = psT.tile([128, 128], BF, name=f"sT{layer}_{b}_{j}", tag="sT")
                        nc.tensor.transpose(out=sT_ps[:], in_=ST[:, j, :], identity=ident_s[:])
                        if j % 2 == 0:
                            nc.vector.tensor_copy(Sblk[:, j, :], sT_ps[:])
                        else:
                            nc.scalar.copy(Sblk[:, j, :], sT_ps[:])
                    nc.sync.dma_start(st_dram[:, c0 * 128:(c0 + Tb) * 128], ST[:].rearrange("p t i -> p (t i)"))
                    nc.scalar.dma_start(s_dram[:, c0 * 128:(c0 + Tb) * 128], Sblk[:].rearrange("p t i -> p (t i)"))
                else:
                    nc.sync.dma_start(ST[:].rearrange("p t i -> p (t i)"), st_dram[:, c0 * 128:(c0 + Tb) * 128])
                    nc.scalar.dma_start(Sblk[:].rearrange("p t i -> p (t i)"), s_dram[:, c0 * 128:(c0 + Tb) * 128])
                # per tile: z = S.T @ xr_block + I @ xl_g  on the PE (no xr gather)
                t = big.tile([128, Tb, 256], BF, name=f"t{layer}_{b}", tag="t", bufs=1)
                for j in range(Tb):
                    z_ps = pz.tile([128, 256], F32, name=f"zps{layer}_{b}_{j}", tag="zps")
                    nc.tensor.matmul(out=z_ps[:], lhsT=Sblk[:, j, :], rhs=xrblk[:], start=True, stop=False)
                    nc.tensor.matmul(out=z_ps[:], lhsT=ident_s[:], rhs=xg[:, j, :], start=False, stop=True)
                    # leaky(z) straight out of PSUM; z itself is never needed again:
                    # sum_e a_e xl[src_e] = num/s because the xr part cancels (sum a = 1)
                    nc.scalar.activation(t[:, j, :], z_ps[:], AF.Prelu, bias=0.0, scale=1.0, alpha=alpha_s[:])
                nc.vector.tensor_tensor(out=t[:], in0=t[:], in1=attf[:, :Tb, :], op=AX.mult)
                # e = per-head sum of t  (strided tree reduce)
                t4 = t[:].rearrange("p t (h c) -> p t h c", c=32)
                r16 = big.tile([128, Tb, 8, 16], BF, name=f"r16_{layer}_{b}", tag="r16", bufs=1)
                nc.vector.tensor_tensor(out=r16[:], in0=t4[:, :, :, 0:16], in1=t4[:, :, :, 16:32], op=AX.add)
                r8 = big.tile([128, Tb, 8, 8], BF, name=f"r8_{layer}_{b}", tag="r8", bufs=1)
                nc.vector.tensor_tensor(out=r8[:], in0=r16[:, :, :, 0:8], in1=r16[:, :, :, 8:16], op=AX.add)
                r4 = big.tile([128, Tb, 8, 4], BF, name=f"r4_{layer}_{b}", tag="r4", bufs=1)
                nc.vector.tensor_tensor(out=r4[:], in0=r8[:, :, :, 0:4], in1=r8[:, :, :, 4:8], op=AX.add)
                r2 = big.tile([128, Tb, 8, 2], BF, name=f"r2_{layer}_{b}", tag="r2", bufs=1)
                nc.vector.tensor_tensor(out=r2[:], in0=r4[:, :, :, 0:2], in1=r4[:, :, :, 2:4], op=AX.add)
                e = big.tile([128, Tb, 8], F32, name=f"e{layer}_{b}", tag="e", bufs=1)
                nc.vector.tensor_tensor(out=e[:], in0=r2[:, :, :, 0], in1=r2[:, :, :, 1], op=AX.add)
                # wp = [xl_g * p | p]   (xr part of the numerator cancels with -xr~)
                wp = big.tile([128, Tb, 264], BF, name=f"wp{layer}_{b}", tag="wp")
                p_sb = big.tile([128, Tb, 8], BF, name=f"p{layer}_{b}", tag="p", bufs=2)
                nc.scalar.activation(p_sb[:], e[:], AF.Exp)
                nc.vector.tensor_copy(wp[:, :, 256:264], p_sb[:])
                nc.vector.tensor_tensor(
                    out=wp[:, :, 0:256].rearrange("p t (h c) -> p t h c", c=32),
                    in0=xg[:].rearrange("p t (h c) -> p t h c", c=32),
                    in1=p_sb[:, :, :, None].to_broadcast([128, Tb, 8, 32]),
                    op=AX.mult)
                out_ps = ps.tile([128, 264], F32, name=f"ops{layer}_{b}", tag="out")
                for j in range(Tb):
                    nc.tensor.matmul(out=out_ps[:], lhsT=ST[:, j, :], rhs=wp[:, j, :],
                                     start=(j == 0), stop=(j == Tb - 1))
                # finalize: outn = num/s ; u = outn - xrhat (+ h_res)
                rec = sb.tile([128, 8], F32, name=f"rec{layer}_{b}", tag="rec")
                nc.vector.reciprocal(rec[:], out_ps[:, 256:264])
                us = res.tile([128, 256], BF, name=f"u{layer}_{b}", tag=f"u{b}")
                nc.vector.tensor_tensor(
                    out=us[:].rearrange("p (h c) -> p h c", c=32),
                    in0=out_ps[:, 0:256].rearrange("p (h c) -> p h c", c=32),
                    in1=rec[:, :, None].to_broadcast([128, 8, 32]), op=AX.mult)
                nc.vector.tensor_add(us[:], us[:], blp_s[layer][:])
                if layer > 0:
                    nc.vector.tensor_add(us[:], us[:], h_sb[b][:])
                sq = sb.tile([128, 256], BF, name=f"sq{layer}_{b}", tag="sq")
                nc.scalar.square(sq[:], us[:])
                nreal = 128 if b < NT - 1 else SH - (NT - 1) * 128
                nc.tensor.matmul(out=st_ps[0:1, 0:256], lhsT=onesc_s[0:nreal, :], rhs=us[0:nreal, :],
                                 start=(b == 0), stop=(b == NT - 1), skip_group_check=True)
                nc.tensor.matmul(out=st_ps[0:1, 256:512], lhsT=onesc_s[0:nreal, :], rhs=sq[0:nreal, :],
                                 start=(b == 0), stop=(b == NT - 1), skip_group_check=True)
                usq[b] = us
            return usq, st_ps

        def bn_tail(layer, usq, st_ps, elu):
            """AllReduce stats, normalize (+ELU); returns h dram + fills h_sb."""
            st_sb = sb.tile([1, 512], F32, name=f"stsb{layer}", tag="stsb", bufs=1)
            nc.vector.tensor_copy(st_sb[:], st_ps[:])
            st_in = dr.tile([1, 512], F32, name=f"stin{layer}")
            st_out = dr.tile([1, 512], F32, name=f"stout{layer}", addr_space="Shared")
            nc.gpsimd.dma_start(st_in[:], st_sb[:])
            nc.gpsimd.collective_compute(
                "AllReduce", AX.add, replica_groups=rg,
                ins=[st_in.opt()], outs=[st_out.opt()])
            st2 = sb.tile([1, 512], F32, name=f"st2{layer}", tag="stsb", bufs=1)
            nc.gpsimd.dma_start(st2[:], st_out[:])

            gi = 0 if layer == 0 else 1
            ab = sb.tile([1, 512], F32, name=f"ab{layer}", tag="ab", bufs=1)   # A | B
            mu = sb.tile([1, 256], F32, name=f"mu{layer}", tag="mu", bufs=1)
            nc.vector.tensor_scalar_mul(mu[:], st2[:, 0:256], 1.0 / N)
            var = sb.tile([1, 256], F32, name=f"var{layer}", tag="var", bufs=1)
            nc.vector.tensor_scalar_mul(var[:], st2[:, 256:512], 1.0 / N)
            mu2 = sb.tile([1, 256], F32, name=f"mu2{layer}", tag="mu2", bufs=1)
            nc.vector.tensor_tensor(out=mu2[:], in0=mu[:], in1=mu[:], op=AX.mult)
            nc.vector.tensor_sub(var[:], var[:], mu2[:])
            nc.vector.tensor_scalar_add(var[:], var[:], EPS)
            # rsqrt = exp(-0.5 * ln(var))  (stays in the ln/exp ACT table set)
            lnv = sb.tile([1, 256], F32, name=f"lnv{layer}", tag="lnv", bufs=1)
            nc.scalar.activation(lnv[:], var[:], AF.Ln)
            rs = sb.tile([1, 256], F32, name=f"rs{layer}", tag="rs", bufs=1)
            nc.scalar.activation(rs[:], lnv[:], AF.Exp, bias=0.0, scale=-0.5)
            nc.vector.tensor_tensor(out=ab[:, 0:256], in0=rs[:], in1=bng_s[gi][:], op=AX.mult)
            nc.vector.tensor_tensor(out=mu2[:], in0=mu[:], in1=ab[:, 0:256], op=AX.mult)
            nc.vector.tensor_tensor(out=ab[:, 256:512], in0=bnb_s[gi][:], in1=mu2[:], op=AX.subtract)
            ab_bc = sb.tile([128, 512], F32, name=f"abbc{layer}", tag="abbc", bufs=1)
            nc.gpsimd.partition_broadcast(ab_bc[:], ab[:])

            for b in range(NT):
                y = res.tile([128, 256], BF, name=f"h{layer}_{b}", tag=f"h{layer % 2}_{b}")
                nc.vector.tensor_tensor(out=y[:], in0=usq[b][:], in1=ab_bc[:, 0:256], op=AX.mult)
                nc.vector.tensor_add(y[:], y[:], ab_bc[:, 256:512])
                if elu:
                    ymin = sb.tile([128, 256], BF, name=f"ymin{layer}_{b}", tag="ymin")
                    nc.vector.tensor_scalar_min(ymin[:], y[:], 0.0)
                    expn = sb.tile([128, 256], BF, name=f"expn{layer}_{b}", tag="expn")
                    nc.scalar.activation(expn[:], ymin[:], AF.Exp)
                    nc.scalar.activation(y[:], y[:], AF.Relu)
                    nc.vector.tensor_add(y[:], y[:], expn[:])
                    nc.vector.tensor_scalar_add(y[:], y[:], -1.0)
                h_sb[b] = y
            return None

        # ---------------- main network ----------------
        st_dram = dr.tile([128, TC * 128], BF, name="st_dram")
        s_dram = dr.tile([128, TC * 128], BF, name="s_dram")
        kdim = NFEAT
        for layer in range(NLAYERS):
            xl_full, xr_tb = node_transform(layer, h0s_d if layer == 0 else None, kdim)
            usq, st_ps = edge_phase(layer, xl_full, xr_tb, st_dram, s_dram)
            bn_tail(layer, usq, st_ps, elu=(layer > 0))
            kdim = 256

        # ---------------- head ----------------
        for nt in range(NT):
            r0 = nt * 128
            hT = []
            for k in range(2):
                t = sb.tile([128, 128], BF, name=f"hTh_{nt}_{k}", tag=f"hT{k}")
                hT_ps = psT.tile([128, 128], BF, name=f"hThp_{nt}_{k}", tag="sT")
                nc.tensor.transpose(out=hT_ps[:], in_=h_sb[nt][:, k * 128:(k + 1) * 128],
                                    identity=ident_s[:])
                if k % 2 == 0:
                    nc.vector.tensor_copy(t[:], hT_ps[:])
                else:
                    nc.scalar.copy(t[:], hT_ps[:])
                hT.append(t)
            y1ps = ps.tile([128, 32], F32, name=f"y1ps{nt}", tag="out")
            for k in range(2):
                nc.tensor.matmul(out=y1ps[:], lhsT=hT[k][:], rhs=lin0w_s[:, k * 32:(k + 1) * 32],
                                 start=(k == 0), stop=False)
            nc.tensor.matmul(out=y1ps[:], lhsT=onesr_s[:], rhs=lin0b_s[:], start=False, stop=True)
            y1 = sb.tile([128, 32], BF, name=f"y1_{nt}", tag="y1")
            nc.scalar.copy(y1[:], y1ps[:])
            ymin = sb.tile([128, 32], BF, name=f"hymin{nt}", tag="hymin")
            nc.vector.tensor_scalar_min(ymin[:], y1[:], 0.0)
            expn = sb.tile([128, 32], BF, name=f"hexpn{nt}", tag="hexpn")
            nc.scalar.activation(expn[:], ymin[:], AF.Exp)
            y1e = sb.tile([128, 32], BF, name=f"y1e_{nt}", tag="y1e")
            nc.scalar.activation(y1e[:], y1[:], AF.Relu)
            nc.vector.tensor_add(y1e[:], y1e[:], expn[:])
            nc.vector.tensor_scalar_add(y1e[:], y1e[:], -1.0)
            y1T_ps = ps.tile([32, 128], BF, name=f"y1Tps{nt}", tag="out")
            nc.tensor.transpose(out=y1T_ps[:], in_=y1e[:], identity=ident_s[:])
            y1T = sb.tile([32, 128], BF, name=f"y1T_{nt}", tag="y1T")
            nc.vector.tensor_copy(y1T[:], y1T_ps[:])
            y2ps = ps.tile([128, 10], F32, name=f"y2ps{nt}", tag="out")
            nc.tensor.matmul(out=y2ps[:], lhsT=y1T[:], rhs=lin1w_s[:], start=True, stop=False)
            nc.tensor.matmul(out=y2ps[:], lhsT=onesr_s[:], rhs=lin1b_s[:], start=False, stop=True)
            outf = sb.tile([128, 10], F32, name=f"outf{nt}", tag="outf")
            nc.scalar.copy(outf[:], y2ps[:])
            nc.sync.dma_start(out_d[r0:r0 + 128, :], outf[:])

    nc.finalize()
    return nc


# ----------------------------------------------------------------------------
# host wrapper
# ----------------------------------------------------------------------------

_CACHE = {}


def _prep(inputs):
    x = np.asarray(inputs["x"], np.float32)
    ei = np.asarray(inputs["edge_index"])
    T, off, TC, srcw, dstw, dstl = _preprocess_edges(ei)

    f = lambda k: np.asarray(inputs[k], np.float32)

    # BN0 on the host (depends only on inputs)
    mu = x.mean(0, dtype=np.float64)
    var = ((x.astype(np.float64) - mu) ** 2).mean(0)
    h0 = ((x - mu.astype(np.float32)) / np.sqrt(var + EPS).astype(np.float32)
          * f("norm0_g") + f("norm0_b")).astype(np.float32)

    def pack_w(w):  # [256, 256] -> [128, 512]
        return np.concatenate([w[:128], w[128:]], axis=1)

    wl = [f("conv0_wl")] + [pack_w(f("convs_wl")[i]) for i in range(NCONVS)]
    wr = [f("conv0_wr")] + [pack_w(f("convs_wr")[i]) for i in range(NCONVS)]
    biasr = [(f("conv0_bl") + f("conv0_br"))[None]] + \
            [(f("convs_bl")[i] + f("convs_br")[i])[None] for i in range(NCONVS)]
    blp = [(f("conv0_bl") + f("conv0_bias"))[None]] + \
          [(f("convs_bl")[i] + f("convs_bias")[i])[None] for i in range(NCONVS)]
    att = [f("conv0_att").reshape(1, 256)] + \
          [f("convs_att")[i].reshape(1, 256) for i in range(NCONVS)]

    bf = lambda a: np.ascontiguousarray(a, np.float32).astype(BF16)
    com = {}
    for l in range(NLAYERS):
        com[f"wl{l}"] = bf(wl[l])
        com[f"wr{l}"] = bf(wr[l])
        com[f"biasr{l}"] = bf(biasr[l])
        com[f"blp{l}"] = bf(np.broadcast_to(blp[l], (128, 256)))
        com[f"att{l}"] = bf(np.broadcast_to(att[l], (128, 256)))
    com["bng0"], com["bnb0"] = f("norm1_g")[None].copy(), f("norm1_b")[None].copy()
    com["bng1"], com["bnb1"] = f("norm2_g")[None].copy(), f("norm2_b")[None].copy()
    com["iota"] = bf(np.broadcast_to(np.arange(128, dtype=np.float32)[None], (128, 128)))
    com["ident"] = bf(np.eye(128, dtype=np.float32))
    com["onesc"] = bf(np.ones((128, 1), np.float32))
    com["onesr"] = bf(np.ones((1, 128), np.float32))
    com["lin0w"] = bf(np.concatenate([f("lin0_w")[:128], f("lin0_w")[128:]], axis=1))
    com["lin0b"] = bf(f("lin0_b")[None])
    com["lin1w"] = bf(f("lin1_w"))
    com["lin1b"] = bf(f("lin1_b")[None])

    in_maps = []
    for c in range(NCORES):
        h0c = np.zeros((SHP, NFEAT), np.float32)
        h0c[:SH] = h0[c * SH:(c + 1) * SH]
        m = dict(com)
        m["h0s"] = bf(h0c)
        m["srcw"] = srcw[c]
        m["dstw"] = dstw[c]
        m["dstl"] = dstl[c].astype(BF16)
        in_maps.append(m)
    return (tuple(T), tuple(off), TC), in_maps


def run(inputs, trace=False, trace_kwargs=None, **extra):
    key, in_maps = _prep(inputs)
    if key not in _CACHE:
        _CACHE[key] = _build_program(list(key[0]), list(key[1]), key[2])
    nc = _CACHE[key]
    kw = dict(extra)
    if trace:
        kw.update(trace=True, trace_kwargs=trace_kwargs or {})
    r = run_bass_kernel_spmd(nc, in_maps, list(range(NCORES)), **kw)
    out = np.empty((N, NCLASS), np.float32)
    for c in range(NCORES):
        out[c * SH:(c + 1) * SH] = r.results[c]["out"][:SH]
    return out, r


def kernel(**inputs) -> np.ndarray:
    out, _ = run(inputs, trace=False)
    return out


# revision 23
# speedup vs baseline: 1.6093x; 1.0105x over previous
"""Trainium2 Bass kernel for nn_GAT_skip_forward_15135464751860.

4-layer GATv2 + BN + residual + ELU + linear head on a fixed random graph
(N=20000 nodes, E=480000 edges + self loops), sharded over 8 NeuronCores by
destination node.

Strategy per layer (per core, nodes sharded 2500/core, dst-sorted edges):
  P1  node transform: xl = h @ wl (raw), xr~ = h @ wr + (bl+br) via PE;
      xl shards AllGathered into a full-table in HBM for gathers.
  P2  edge phase per 128-dst block: batched dma_gather of xl[src] and
      xr~[dst] (bf16 rows), z = xl+xr~ on GpSimd, leaky via Prelu on ACT,
      att-dot on DVE (mult + strided tree-reduce), p = exp(e) on ACT,
      wp = [z*p | p], one-hot S_T built by is_equal(dstl, iota), and
      out[i,:] = sum_e S[i,e] wp[e,:] accumulated on the PE into PSUM.
      Softmax denominator rides along as wp column block 256:264.
      out = num/s - xr~ + (bl + conv_bias) (+ residual), since
      sum_e a_e * xl[src_e] = sum_e a_e * z_e - xr~[dst].
  BN  stats via ones-matmul into PSUM, AllReduce, normalize + (ELU).
Head: two small matmuls with a PE transpose in between.
"""
import os, sys

os.environ.setdefault("JAX_PLATFORMS", "")
if "/opt/trn_rl_repo" not in sys.path:
    sys.path.insert(0, "/opt/trn_rl_repo")

import numpy as np
import ml_dtypes
from contextlib import ExitStack

import concourse.bass as bass
import concourse.tile as tile
from concourse import bacc, mybir
from concourse.bass_utils import run_bass_kernel_spmd

BF16 = ml_dtypes.bfloat16

# problem dims
N = 20000
E = 480000
NFEAT = 128
H = 8
C = 32
HC = 256
NCLASS = 10
NCONVS = 3
EPS = 1e-5
SLOPE = 0.2

NCORES = 8
SH = N // NCORES          # 2500 real nodes per core
NT = 20                   # dst blocks / node tiles per core (20*128 = 2560)
SHP = NT * 128            # padded shard rows
NLAYERS = 4

F32 = mybir.dt.float32
BF = mybir.dt.bfloat16
I16 = mybir.dt.int16
AX = mybir.AluOpType
AF = mybir.ActivationFunctionType


# ----------------------------------------------------------------------------
# host-side graph preprocessing
# ----------------------------------------------------------------------------

def _preprocess_edges(edge_index):
    src = np.concatenate([np.asarray(edge_index[0], np.int64), np.arange(N)])
    dst = np.concatenate([np.asarray(edge_index[1], np.int64), np.arange(N)])

    core = dst // SH
    per_core = []
    for c in range(NCORES):
        m = core == c
        s, d = src[m], dst[m]
        o = np.argsort(d, kind="stable")
        s, d = s[o], d[o]
        dloc = d - c * SH
        blk = dloc // 128
        per_core.append((s, dloc, blk))

    # uniform tile count per block across cores (SPMD: one program).
    # need[b] = pad-dst slots that must receive a dummy edge so their softmax
    # denominator stays finite (0 x inf -> NaN otherwise).
    need = np.array([128 - min(128, SH - b * 128) for b in range(NT)], np.int64)
    T = np.zeros(NT, np.int64)
    for c in range(NCORES):
        _, _, blk = per_core[c]
        cnt = np.bincount(blk, minlength=NT)
        T = np.maximum(T, (cnt + need + 127) // 128)
    T = np.maximum(T, 1)
    off = np.zeros(NT + 1, np.int64)
    off[1:] = np.cumsum(T)
    TC = int(off[-1])

    srcw = np.zeros((NCORES, 128, TC * 8), np.int16)
    dstw = np.zeros((NCORES, 128, TC * 8), np.int16)
    dstl = np.full((NCORES, 128, TC), -1.0, np.float32)

    for c in range(NCORES):
        s, dloc, blk = per_core[c]
        for b in range(NT):
            m = blk == b
            sb_, db_ = s[m], dloc[m]
            L = int(T[b]) * 128
            srow = np.zeros(L, np.int64)
            drow = np.zeros(L, np.int64)
            dl = np.full(L, -1.0, np.float32)
            n = len(sb_)
            srow[:n] = (sb_ // SH) * SHP + (sb_ % SH)   # row in AllGathered table
            drow[:n] = db_                              # row in local xr table
            dl[:n] = db_ - b * 128                      # in [0,128)
            nreal = min(128, SH - b * 128)
            for k in range(128 - nreal):                # dummy edges -> pad dsts
                dl[n + k] = nreal + k
            # wrapped int16 idx layout for dma_gather: arr[p, s] = idx[s*16+p],
            # replicated over the 8 Q7 cores (16 partitions * 8)
            sw = srow.reshape(-1, 16).T.astype(np.int16)
            dw = drow.reshape(-1, 16).T.astype(np.int16)
            c0 = int(off[b])
            srcw[c, :, c0 * 8:(c0 + int(T[b])) * 8] = np.tile(sw, (8, 1))
            dstw[c, :, c0 * 8:(c0 + int(T[b])) * 8] = np.tile(dw, (8, 1))
            dstl[c, :, c0:c0 + int(T[b])] = dl.reshape(int(T[b]), 128).T

    return [int(t) for t in T], [int(o) for o in off], TC, srcw, dstw, dstl


# ----------------------------------------------------------------------------
# device program
# ----------------------------------------------------------------------------

def _build_program(T, off, TC):
    TMAX = max(T)
    nc = bacc.Bacc(None, target_bir_lowering=False, num_devices=NCORES)

    # --- external inputs -----------------------------------------------------
    h0s_d = nc.dram_tensor("h0s", [SHP, NFEAT], BF, kind="ExternalInput")
    srcw_d = nc.dram_tensor("srcw", [128, TC * 8], I16, kind="ExternalInput")
    dstw_d = nc.dram_tensor("dstw", [128, TC * 8], I16, kind="ExternalInput")
    dstl_d = nc.dram_tensor("dstl", [128, TC], BF, kind="ExternalInput")

    wl_d = [nc.dram_tensor(f"wl{l}", [128, 256 if l == 0 else 512], BF, kind="ExternalInput") for l in range(NLAYERS)]
    wr_d = [nc.dram_tensor(f"wr{l}", [128, 256 if l == 0 else 512], BF, kind="ExternalInput") for l in range(NLAYERS)]
    biasr_d = [nc.dram_tensor(f"biasr{l}", [1, 256], BF, kind="ExternalInput") for l in range(NLAYERS)]
    blp_d = [nc.dram_tensor(f"blp{l}", [128, 256], BF, kind="ExternalInput") for l in range(NLAYERS)]
    att_d = [nc.dram_tensor(f"att{l}", [128, 256], BF, kind="ExternalInput") for l in range(NLAYERS)]
    bng_d = [nc.dram_tensor(f"bng{s}", [1, 256], F32, kind="ExternalInput") for s in range(2)]
    bnb_d = [nc.dram_tensor(f"bnb{s}", [1, 256], F32, kind="ExternalInput") for s in range(2)]
    iota_d = nc.dram_tensor("iota", [128, 128], BF, kind="ExternalInput")
    ident_d = nc.dram_tensor("ident", [128, 128], BF, kind="ExternalInput")
    onesc_d = nc.dram_tensor("onesc", [128, 1], BF, kind="ExternalInput")
    onesr_d = nc.dram_tensor("onesr", [1, 128], BF, kind="ExternalInput")
    lin0w_d = nc.dram_tensor("lin0w", [128, 2 * 32], BF, kind="ExternalInput")
    lin0b_d = nc.dram_tensor("lin0b", [1, 32], BF, kind="ExternalInput")
    lin1w_d = nc.dram_tensor("lin1w", [32, 10], BF, kind="ExternalInput")
    lin1b_d = nc.dram_tensor("lin1b", [1, 10], BF, kind="ExternalInput")

    out_d = nc.dram_tensor("out", [SHP, NCLASS], F32, kind="ExternalOutput")

    rg = [list(range(NCORES))]

    with tile.TileContext(nc) as tc, ExitStack() as ctx:
        cst = ctx.enter_context(tc.tile_pool(name="cst", bufs=1))
        sb = ctx.enter_context(tc.tile_pool(name="sb", bufs=2))
        big = ctx.enter_context(tc.tile_pool(name="big", bufs=2))
        res = ctx.enter_context(tc.tile_pool(name="res", bufs=1))
        ps = ctx.enter_context(tc.tile_pool(name="ps", bufs=2, space="PSUM"))
        pstat = ctx.enter_context(tc.tile_pool(name="pstat", bufs=1, space="PSUM"))
        pz = ctx.enter_context(tc.tile_pool(name="pz", bufs=4, space="PSUM"))
        psT = ctx.enter_context(tc.tile_pool(name="psT", bufs=1, space="PSUM"))
        # ps2 + pstat1 + pz4 + psT1 = 8 PSUM banks
        dr = ctx.enter_context(tc.tile_pool(name="dr", bufs=1, space="DRAM"))

        # --- load constants --------------------------------------------------
        _cl = [0]
        def cload(dram, shape, dtype, name):
            t = cst.tile(shape, dtype, name=name)
            eng = nc.sync if _cl[0] % 2 == 0 else nc.scalar
            _cl[0] += 1
            eng.dma_start(t[:], dram[:])
            return t

        dstl_s = cload(dstl_d, [128, TC], BF, "dstl_s")
        wl_s = [cload(wl_d[l], [128, 256 if l == 0 else 512], BF, f"wl_s{l}") for l in range(NLAYERS)]
        wr_s = [cload(wr_d[l], [128, 256 if l == 0 else 512], BF, f"wr_s{l}") for l in range(NLAYERS)]
        biasr_s = [cload(biasr_d[l], [1, 256], BF, f"biasr_s{l}") for l in range(NLAYERS)]
        blp_s = [cload(blp_d[l], [128, 256], BF, f"blp_s{l}") for l in range(NLAYERS)]
        att_s = [cload(att_d[l], [128, 256], BF, f"att_s{l}") for l in range(NLAYERS)]
        bng_s = [cload(bng_d[s], [1, 256], F32, f"bng_s{s}") for s in range(2)]
        bnb_s = [cload(bnb_d[s], [1, 256], F32, f"bnb_s{s}") for s in range(2)]
        iota_s = cload(iota_d, [128, 128], BF, "iota_s")
        ident_s = cload(ident_d, [128, 128], BF, "ident_s")
        onesc_s = cload(onesc_d, [128, 1], BF, "onesc_s")
        onesr_s = cload(onesr_d, [1, 128], BF, "onesr_s")
        lin0w_s = cload(lin0w_d, [128, 64], BF, "lin0w_s")
        lin0b_s = cload(lin0b_d, [1, 32], BF, "lin0b_s")
        lin1w_s = cload(lin1w_d, [32, 10], BF, "lin1w_s")
        lin1b_s = cload(lin1b_d, [1, 10], BF, "lin1b_s")
        alpha_s = cst.tile([128, 1], F32, name="alpha_s")
        nc.vector.memset(alpha_s[:], SLOPE)

        h_sb = [None] * NT       # SBUF resident h tiles (residual input)

        def node_transform(layer, h_dram, kdim):
            """xl/xr tables for this layer from h_dram [SHP, kdim] (layer 0)
            or from resident h_sb tiles (later layers, on-chip transpose)."""
            xl_sh = dr.tile([SHP, 256], BF, name=f"xl_sh{layer}")
            xr_tb = dr.tile([SHP, 256], BF, name=f"xr_tb{layer}")
            nhalf = kdim // 128
            for nt in range(NT):
                r0 = nt * 128
                hT = []
                for k in range(nhalf):
                    t = sb.tile([128, 128], BF, name=f"hT{layer}_{nt}_{k}", tag=f"hT{k}")
                    if layer == 0:
                        eng = nc.sync if (nt % 2 == 0) else nc.scalar
                        eng.dma_start(out=t[:], in_=h_dram[r0:r0 + 128, k * 128:(k + 1) * 128], transpose=True)
                    else:
                        hT_ps = psT.tile([128, 128], BF, name=f"hTp{layer}_{nt}_{k}", tag="sT")
                        nc.tensor.transpose(out=hT_ps[:], in_=h_sb[nt][:, k * 128:(k + 1) * 128],
                                            identity=ident_s[:])
                        if k % 2 == 0:
                            nc.vector.tensor_copy(t[:], hT_ps[:])
                        else:
                            nc.scalar.copy(t[:], hT_ps[:])
                    hT.append(t)
                xl_ps = pz.tile([128, 256], F32, name=f"xlps{layer}_{nt}", tag="zps")
                for k in range(nhalf):
                    nc.tensor.matmul(out=xl_ps[:], lhsT=hT[k][:], rhs=wl_s[layer][:, k * 256:(k + 1) * 256],
                                     start=(k == 0), stop=(k == nhalf - 1))
                xl_sb = sb.tile([128, 256], BF, name=f"xlsb{layer}_{nt}", tag="xlsb")
                nc.scalar.copy(xl_sb[:], xl_ps[:])
                nc.sync.dma_start(xl_sh[r0:r0 + 128, :], xl_sb[:])

                xr_ps = pz.tile([128, 256], F32, name=f"xrps{layer}_{nt}", tag="zps")
                for k in range(nhalf):
                    nc.tensor.matmul(out=xr_ps[:], lhsT=hT[k][:], rhs=wr_s[layer][:, k * 256:(k + 1) * 256],
                                     start=(k == 0), stop=False)
                nc.tensor.matmul(out=xr_ps[:], lhsT=onesr_s[:], rhs=biasr_s[layer][:],
                                 start=False, stop=True)
                xr_sb = sb.tile([128, 256], BF, name=f"xrsb{layer}_{nt}", tag="xrsb")
                nc.scalar.copy(xr_sb[:], xr_ps[:])
                nc.sync.dma_start(xr_tb[r0:r0 + 128, :], xr_sb[:])

            xl_full = dr.tile([NCORES * SHP, 256], BF, name=f"xl_full{layer}", addr_space="Shared")
            nc.gpsimd.collective_compute(
                "AllGather", AX.bypass, replica_groups=rg,
                ins=[xl_sh.opt()], outs=[xl_full.opt()])
            return xl_full, xr_tb

        def edge_phase(layer, xl_full, xr_tb, st_dram, s_dram):
            """GATv2 aggregation; returns list of usq tiles ([128,512]: u|u^2)
            and the stats psum tile."""
            st_ps = pstat.tile([1, 512], F32, name=f"stats{layer}", tag="stats")
            # materialize att without a step-0 broadcast (keeps DVE 2x mode)
            attf = cst.tile([128, TMAX, 256], BF, name=f"attf{layer}", tag="attf")
            nc.vector.tensor_copy(
                attf[:], att_s[layer][:, None, :].to_broadcast([128, TMAX, 256]))
            usq = [None] * NT
            for b in range(NT):
                Tb = T[b]
                L = Tb * 128
                c0 = off[b]
                siw = sb.tile([128, Tb * 8], I16, name=f"siw{layer}_{b}", tag="siw", bufs=3)
                nc.sync.dma_start(siw[:], srcw_d[:, c0 * 8:(c0 + Tb) * 8])
                xg = big.tile([128, Tb, 256], BF, name=f"xg{layer}_{b}", tag="z", bufs=3)
                nc.gpsimd.dma_gather(
                    out_ap=xg[:], in_ap=xl_full[:], idxs_ap=siw[:],
                    num_idxs=L, num_idxs_reg=L, elem_size=256, single_packet=False)
                xrblk = sb.tile([128, 256], BF, name=f"xrblk{layer}_{b}", tag="xrblk")
                nc.sync.dma_start(xrblk[:], xr_tb[b * 128:(b + 1) * 128, :])
                ST = big.tile([128, Tb, 128], BF, name=f"ST{layer}_{b}", tag="ST")
                Sblk = big.tile([128, Tb, 128], BF, name=f"Sblk{layer}_{b}", tag="Sblk")
                if layer == 0:
                    # build one-hots once: S_T[e, i] = (dstl[e] == i); S = transpose
                    nc.vector.tensor_tensor(
                        out=ST[:],
                        in0=dstl_s[:, c0:c0 + Tb, None].to_broadcast([128, Tb, 128]),
                        in1=iota_s[:, None, :].to_broadcast([128, Tb, 128]),
                        op=AX.is_equal)
                    for j in range(Tb):
                        sT_ps = psT.tile([128, 128], BF, name=f"sT{layer}_{b}_{j}", tag="sT")
                        nc.tensor.transpose(out=sT_ps[:], in_=ST[:, j, :], identity=ident_s[:])
                        if j % 2 == 0:
                            nc.vector.tensor_copy(Sblk[:, j, :], sT_ps[:])
                        else:
                            nc.scalar.copy(Sblk[:, j, :], sT_ps[:])
                    nc.sync.dma_start(st_dram[:, c0 * 128:(c0 + Tb) * 128], ST[:].rearrange("p t i -> p (t i)"))
                    nc.scalar.dma_start(s_dram[:, c0 * 128:(c0 + Tb) * 128], Sblk[:].rearrange("p t i -> p (t i)"))
                else:
                    nc.sync.dma_start(ST[:].rearrange("p t i -> p (t i)"), st_dram[:, c0 * 128:(c0 + Tb) * 128])
                    nc.scalar.dma_start(Sblk[:].rearrange("p t i -> p (t i)"), s_dram[:, c0 * 128:(c0 + Tb) * 128])
                # per tile: z = S.T @ xr_block + I @ xl_g  on the PE (no xr gather)
                t = big.tile([128, Tb, 256], BF, name=f"t{layer}_{b}", tag="t", bufs=1)
                for j in range(Tb):
                    z_ps = pz.tile([128, 256], F32, name=f"zps{layer}_{b}_{j}", tag="zps")
                    nc.tensor.matmul(out=z_ps[:], lhsT=Sblk[:, j, :], rhs=xrblk[:], start=True, stop=False)
                    nc.tensor.matmul(out=z_ps[:], lhsT=ident_s[:], rhs=xg[:, j, :], start=False, stop=True)
                    # leaky(z) straight out of PSUM; z itself is never needed again:
                    # sum_e a_e xl[src_e] = num/s because the xr part cancels (sum a = 1)
                    nc.scalar.activation(t[:, j, :], z_ps[:], AF.Prelu, bias=0.0, scale=1.0, alpha=alpha_s[:])
                nc.vector.tensor_tensor(out=t[:], in0=t[:], in1=attf[:, :Tb, :], op=AX.mult)
                # e = per-head sum of t  (strided tree reduce)
                t4 = t[:].rearrange("p t (h c) -> p t h c", c=32)
                r16 = big.tile([128, Tb, 8, 16], BF, name=f"r16_{layer}_{b}", tag="r16", bufs=1)
                nc.vector.tensor_tensor(out=r16[:], in0=t4[:, :, :, 0:16], in1=t4[:, :, :, 16:32], op=AX.add)
                r8 = big.tile([128, Tb, 8, 8], BF, name=f"r8_{layer}_{b}", tag="r8", bufs=1)
                nc.vector.tensor_tensor(out=r8[:], in0=r16[:, :, :, 0:8], in1=r16[:, :, :, 8:16], op=AX.add)
                r4 = big.tile([128, Tb, 8, 4], BF, name=f"r4_{layer}_{b}", tag="r4", bufs=1)
                nc.vector.tensor_tensor(out=r4[:], in0=r8[:, :, :, 0:4], in1=r8[:, :, :, 4:8], op=AX.add)
                r2 = big.tile([128, Tb, 8, 2], BF, name=f"r2_{layer}_{b}", tag="r2", bufs=1)
                nc.vector.tensor_tensor(out=r2[:], in0=r4[:, :, :, 0:2], in1=r4[:, :, :, 2:4], op=AX.add)
                e = big.tile([128, Tb, 8], F32, name=f"e{layer}_{b}", tag="e", bufs=1)
                nc.vector.tensor_tensor(out=e[:], in0=r2[:, :, :, 0], in1=r2[:, :, :, 1], op=AX.add)
                # wp = [xl_g * p | p]   (xr part of the numerator cancels with -xr~)
                wp = big.tile([128, Tb, 264], BF, name=f"wp{layer}_{b}", tag="wp")
                p_sb = big.tile([128, Tb, 8], BF, name=f"p{layer}_{b}", tag="p", bufs=2)
                nc.scalar.activation(p_sb[:], e[:], AF.Exp)
                nc.vector.tensor_copy(wp[:, :, 256:264], p_sb[:])
                nc.vector.tensor_tensor(
                    out=wp[:, :, 0:256].rearrange("p t (h c) -> p t h c", c=32),
                    in0=xg[:].rearrange("p t (h c) -> p t h c", c=32),
                    in1=p_sb[:, :, :, None].to_broadcast([128, Tb, 8, 32]),
                    op=AX.mult)
                out_ps = ps.tile([128, 264], F32, name=f"ops{layer}_{b}", tag="out")
                for j in range(Tb):
                    nc.tensor.matmul(out=out_ps[:], lhsT=ST[:, j, :], rhs=wp[:, j, :],
                                     start=(j == 0), stop=(j == Tb - 1))
                # finalize: outn = num/s ; u = outn - xrhat (+ h_res)
                rec = sb.tile([128, 8], F32, name=f"rec{layer}_{b}", tag="rec")
                nc.vector.reciprocal(rec[:], out_ps[:, 256:264])
                us = res.tile([128, 256], BF, name=f"u{layer}_{b}", tag=f"u{b}")
                nc.vector.tensor_tensor(
                    out=us[:].rearrange("p (h c) -> p h c", c=32),
                    in0=out_ps[:, 0:256].rearrange("p (h c) -> p h c", c=32),
                    in1=rec[:, :, None].to_broadcast([128, 8, 32]), op=AX.mult)
                nc.vector.tensor_add(us[:], us[:], blp_s[layer][:])
                if layer > 0:
                    nc.vector.tensor_add(us[:], us[:], h_sb[b][:])
                sq = sb.tile([128, 256], BF, name=f"sq{layer}_{b}", tag="sq")
                nc.scalar.square(sq[:], us[:])
                nreal = 128 if b < NT - 1 else SH - (NT - 1) * 128
                nc.tensor.matmul(out=st_ps[0:1, 0:256], lhsT=onesc_s[0:nreal, :], rhs=us[0:nreal, :],
                                 start=(b == 0), stop=(b == NT - 1), skip_group_check=True)
                nc.tensor.matmul(out=st_ps[0:1, 256:512], lhsT=onesc_s[0:nreal, :], rhs=sq[0:nreal, :],
                                 start=(b == 0), stop=(b == NT - 1), skip_group_check=True)
                usq[b] = us
            return usq, st_ps

        def bn_tail(layer, usq, st_ps, elu):
            """AllReduce stats, normalize (+ELU); returns h dram + fills h_sb."""
            st_sb = sb.tile([1, 512], F32, name=f"stsb{layer}", tag="stsb", bufs=1)
            nc.vector.tensor_copy(st_sb[:], st_ps[:])
            st_in = dr.tile([1, 512], F32, name=f"stin{layer}")
            st_out = dr.tile([1, 512], F32, name=f"stout{layer}", addr_space="Shared")
            nc.gpsimd.dma_start(st_in[:], st_sb[:])
            nc.gpsimd.collective_compute(
                "AllReduce", AX.add, replica_groups=rg,
                ins=[st_in.opt()], outs=[st_out.opt()])
            st2 = sb.tile([1, 512], F32, name=f"st2{layer}", tag="stsb", bufs=1)
            nc.gpsimd.dma_start(st2[:], st_out[:])

            gi = 0 if layer == 0 else 1
            ab = sb.tile([1, 512], F32, name=f"ab{layer}", tag="ab", bufs=1)   # A | B
            mu = sb.tile([1, 256], F32, name=f"mu{layer}", tag="mu", bufs=1)
            nc.vector.tensor_scalar_mul(mu[:], st2[:, 0:256], 1.0 / N)
            var = sb.tile([1, 256], F32, name=f"var{layer}", tag="var", bufs=1)
            nc.vector.tensor_scalar_mul(var[:], st2[:, 256:512], 1.0 / N)
            mu2 = sb.tile([1, 256], F32, name=f"mu2{layer}", tag="mu2", bufs=1)
            nc.vector.tensor_tensor(out=mu2[:], in0=mu[:], in1=mu[:], op=AX.mult)
            nc.vector.tensor_sub(var[:], var[:], mu2[:])
            nc.vector.tensor_scalar_add(var[:], var[:], EPS)
            # rsqrt = exp(-0.5 * ln(var))  (stays in the ln/exp ACT table set)
            lnv = sb.tile([1, 256], F32, name=f"lnv{layer}", tag="lnv", bufs=1)
            nc.scalar.activation(lnv[:], var[:], AF.Ln)
            rs = sb.tile([1, 256], F32, name=f"rs{layer}", tag="rs", bufs=1)
            nc.scalar.activation(rs[:], lnv[:], AF.Exp, bias=0.0, scale=-0.5)
            nc.vector.tensor_tensor(out=ab[:, 0:256], in0=rs[:], in1=bng_s[gi][:], op=AX.mult)
            nc.vector.tensor_tensor(out=mu2[:], in0=mu[:], in1=ab[:, 0:256], op=AX.mult)
            nc.vector.tensor_tensor(out=ab[:, 256:512], in0=bnb_s[gi][:], in1=mu2[:], op=AX.subtract)
            ab_bc = sb.tile([128, 512], F32, name=f"abbc{layer}", tag="abbc", bufs=1)
            nc.gpsimd.partition_broadcast(ab_bc[:], ab[:])

            for b in range(NT):
                y = res.tile([128, 256], BF, name=f"h{layer}_{b}", tag=f"h{layer % 2}_{b}")
                nc.vector.tensor_tensor(out=y[:], in0=usq[b][:], in1=ab_bc[:, 0:256], op=AX.mult)
                nc.vector.tensor_add(y[:], y[:], ab_bc[:, 256:512])
                if elu:
                    ymin = sb.tile([128, 256], BF, name=f"ymin{layer}_{b}", tag="ymin")
                    nc.vector.tensor_scalar_min(ymin[:], y[:], 0.0)
                    expn = sb.tile([128, 256], BF, name=f"expn{layer}_{b}", tag="expn")
                    nc.scalar.activation(expn[:], ymin[:], AF.Exp)
                    nc.scalar.activation(y[:], y[:], AF.Relu)
                    nc.vector.tensor_add(y[:], y[:], expn[:])
                    nc.vector.tensor_scalar_add(y[:], y[:], -1.0)
                h_sb[b] = y
            return None

        # ---------------- main network ----------------
        st_dram = dr.tile([128, TC * 128], BF, name="st_dram")
        s_dram = dr.tile([128, TC * 128], BF, name="s_dram")
        kdim = NFEAT
        for layer in range(NLAYERS):
            xl_full, xr_tb = node_transform(layer, h0s_d if layer == 0 else None, kdim)
            usq, st_ps = edge_phase(layer, xl_full, xr_tb, st_dram, s_dram)
            bn_tail(layer, usq, st_ps, elu=(layer > 0))
            kdim = 256

        # ---------------- head ----------------
        for nt in range(NT):
            r0 = nt * 128
            hT = []
            for k in range(2):
                t = sb.tile([128, 128], BF, name=f"hTh_{nt}_{k}", tag=f"hT{k}")
                hT_ps = psT.tile([128, 128], BF, name=f"hThp_{nt}_{k}", tag="sT")
                nc.tensor.transpose(out=hT_ps[:], in_=h_sb[nt][:, k * 128:(k + 1) * 128],
                                    identity=ident_s[:])
                if k % 2 == 0:
                    nc.vector.tensor_copy(t[:], hT_ps[:])
                else:
                    nc.scalar.copy(t[:], hT_ps[:])
                hT.append(t)
            y1ps = ps.tile([128, 32], F32, name=f"y1ps{nt}", tag="out")
            for k in range(2):
                nc.tensor.matmul(out=y1ps[:], lhsT=hT[k][:], rhs=lin0w_s[:, k * 32:(k + 1) * 32],
                                 start=(k == 0), stop=False)
            nc.tensor.matmul(out=y1ps[:], lhsT=onesr_s[:], rhs=lin0b_s[:], start=False, stop=True)
            y1 = sb.tile([128, 32], BF, name=f"y1_{nt}", tag="y1")
            nc.scalar.copy(y1[:], y1ps[:])
            ymin = sb.tile([128, 32], BF, name=f"hymin{nt}", tag="hymin")
            nc.vector.tensor_scalar_min(ymin[:], y1[:], 0.0)
            expn = sb.tile([128, 32], BF, name=f"hexpn{nt}", tag="hexpn")
            nc.scalar.activation(expn[:], ymin[:], AF.Exp)
            y1e = sb.tile([128, 32], BF, name=f"y1e_{nt}", tag="y1e")
            nc.scalar.activation(y1e[:], y1[:], AF.Relu)
            nc.vector.tensor_add(y1e[:], y1e[:], expn[:])
            nc.vector.tensor_scalar_add(y1e[:], y1e[:], -1.0)
            y1T_ps = ps.tile([32, 128], BF, name=f"y1Tps{nt}", tag="out")
            nc.tensor.transpose(out=y1T_ps[:], in_=y1e[:], identity=ident_s[:])
            y1T = sb.tile([32, 128], BF, name=f"y1T_{nt}", tag="y1T")
            nc.vector.tensor_copy(y1T[:], y1T_ps[:])
            y2ps = ps.tile([128, 10], F32, name=f"y2ps{nt}", tag="out")
            nc.tensor.matmul(out=y2ps[:], lhsT=y1T[:], rhs=lin1w_s[:], start=True, stop=False)
            nc.tensor.matmul(out=y2ps[:], lhsT=onesr_s[:], rhs=lin1b_s[:], start=False, stop=True)
            outf = sb.tile([128, 10], F32, name=f"outf{nt}", tag="outf")
            nc.scalar.copy(outf[:], y2ps[:])
            nc.sync.dma_start(out_d[r0:r0 + 128, :], outf[:])

    nc.finalize()
    return nc


# ----------------------------------------------------------------------------
# host wrapper
# ----------------------------------------------------------------------------

_CACHE = {}


def _prep(inputs):
    x = np.asarray(inputs["x"], np.float32)
    ei = np.asarray(inputs["edge_index"])
    T, off, TC, srcw, dstw, dstl = _preprocess_edges(ei)

    f = lambda k: np.asarray(inputs[k], np.float32)

    # BN0 on the host (depends only on inputs)
    mu = x.mean(0, dtype=np.float64)
    var = ((x.astype(np.float64) - mu) ** 2).mean(0)
    h0 = ((x - mu.astype(np.float32)) / np.sqrt(var + EPS).astype(np.float32)
          * f("norm0_g") + f("norm0_b")).astype(np.float32)

    def pack_w(w):  # [256, 256] -> [128, 512]
        return np.concatenate([w[:128], w[128:]], axis=1)

    wl = [f("conv0_wl")] + [pack_w(f("convs_wl")[i]) for i in range(NCONVS)]
    wr = [f("conv0_wr")] + [pack_w(f("convs_wr")[i]) for i in range(NCONVS)]
    biasr = [(f("conv0_bl") + f("conv0_br"))[None]] + \
            [(f("convs_bl")[i] + f("convs_br")[i])[None] for i in range(NCONVS)]
    blp = [(f("conv0_bl") + f("conv0_bias"))[None]] + \
          [(f("convs_bl")[i] + f("convs_bias")[i])[None] for i in range(NCONVS)]
    att = [f("conv0_att").reshape(1, 256)] + \
          [f("convs_att")[i].reshape(1, 256) for i in range(NCONVS)]

    bf = lambda a: np.ascontiguousarray(a, np.float32).astype(BF16)
    com = {}
    for l in range(NLAYERS):
        com[f"wl{l}"] = bf(wl[l])
        com[f"wr{l}"] = bf(wr[l])
        com[f"biasr{l}"] = bf(biasr[l])
        com[f"blp{l}"] = bf(np.broadcast_to(blp[l], (128, 256)))
        com[f"att{l}"] = bf(np.broadcast_to(att[l], (128, 256)))
    com["bng0"], com["bnb0"] = f("norm1_g")[None].copy(), f("norm1_b")[None].copy()
    com["bng1"], com["bnb1"] = f("norm2_g")[None].copy(), f("norm2_b")[None].copy()
    com["iota"] = bf(np.broadcast_to(np.arange(128, dtype=np.float32)[None], (128, 128)))
    com["ident"] = bf(np.eye(128, dtype=np.float32))
    com["onesc"] = bf(np.ones((128, 1), np.float32))
    com["onesr"] = bf(np.ones((1, 128), np.float32))
    com["lin0w"] = bf(np.concatenate([f("lin0_w")[:128], f("lin0_w")[128:]], axis=1))
    com["lin0b"] = bf(f("lin0_b")[None])
    com["lin1w"] = bf(f("lin1_w"))
    com["lin1b"] = bf(f("lin1_b")[None])

    in_maps = []
    for c in range(NCORES):
        h0c = np.zeros((SHP, NFEAT), np.float32)
        h0c[:SH] = h0[c * SH:(c + 1) * SH]
        m = dict(com)
        m["h0s"] = bf(h0c)
        m["srcw"] = srcw[c]
        m["dstw"] = dstw[c]
        m["dstl"] = dstl[c].astype(BF16)
        in_maps.append(m)
    return (tuple(T), tuple(off), TC), in_maps


def run(inputs, trace=False, trace_kwargs=None, **extra):
    key, in_maps = _prep(inputs)
    if key not in _CACHE:
        _CACHE[key] = _build_program(list(key[0]), list(key[1]), key[2])
    nc = _CACHE[key]
    kw = dict(extra)
    if trace:
        kw.update(trace=True, trace_kwargs=trace_kwargs or {})
    r = run_bass_kernel_spmd(nc, in_maps, list(range(NCORES)), **kw)
    out = np.empty((N, NCLASS), np.float32)
    for c in range(NCORES):
        out[c * SH:(c + 1) * SH] = r.results[c]["out"][:SH]
    return out, r


def kernel(**inputs) -> np.ndarray:
    out, _ = run(inputs, trace=False)
    return out
